# revision 1
# baseline (speedup 1.0000x reference)
"""Trainium2 Bass kernel for nn_CommonFeatureExtractor.

Data-parallel over 8 NeuronCores: batch dim (4096) sharded into 8 x 512,
weights replicated. Inside each core everything is computed in the
"transposed" layout [feature_on_partitions, batch_free] so that all matmul
contractions (which run over the partition axis on the PE) need no on-chip
transposes: the host feeds x already transposed and the weights are natural
[din, dout] = [K, M] layout, which is exactly what the PE's lhsT wants.

Pipeline per core (B=512 samples):
  A) 5 encoder MLPs (fp32 data, fp32r matmuls), fps.T stored bf16 [128,20,512]
  B) stats: pair products/squares (bf16) -> PE ones-matmul partition
     reductions -> d[10,B], ss[5,B]; softmax over selected pairs via
     ln/exp trick; per-pair weights broadcast to [128,B] via K=1 matmuls
  C) masked aggregation: G_i = sum_{pairs p containing i} (prod_p>0)*wq_p
     (+ mean-fallback), common.T = sum_i fps_i.T * G_i; wsum.T likewise with
     learned softmax gate weights
  D) enhance (sigmoid gate) + fuse matmuls -> fused.T [512, 512] -> host
     transposes back and concatenates.
"""

import numpy as np

import concourse.bass as bass
import concourse.mybir as mybir
import concourse.tile as tile
from concourse import bacc
from concourse.bass_utils import run_bass_kernel_spmd

F32 = mybir.dt.float32
F32R = mybir.dt.float32r
BF16 = mybir.dt.bfloat16
FP16 = mybir.dt.float16
ALU = mybir.AluOpType
AF = mybir.ActivationFunctionType

N_CORES = 8
B = 4096
BC = B // N_CORES  # 512 samples per core
H = 512
P = 128

AP_D, MA_D, MB_D, MC_D, PH_D = 2048, 167, 2048, 2048, 27
# encoders: (name, din, padded K tiles, hidden dh, M tiles = dh/128)
ENCS = [
    ("ap", AP_D, 16, 512),
    ("ma", MA_D, 2, 256),
    ("mb", MB_D, 16, 512),
    ("mc", MC_D, 16, 512),
    ("ph", PH_D, 1, 128),
]
XT_K = sum(e[2] for e in ENCS)  # 51 padded k-tiles of x
XT_OFF = np.cumsum([0] + [e[2] for e in ENCS])[:-1]  # [0,16,18,34,50]

_I = [0, 0, 0, 0, 1, 1, 1, 2, 2, 3]
_J = [1, 2, 3, 4, 2, 3, 4, 3, 4, 4]
PAIR_IDX = {(_I[p], _J[p]): p for p in range(10)}
# compute order: small encoders first so most pair-stats overlap phase A
ORDER = ["ma", "ph", "ap", "mb", "mc"]
ENC_BY_NAME = {e[0]: (i, e) for i, e in enumerate(ENCS)}
# pairs containing encoder i
PAIRS_OF = [[p for p in range(10) if _I[p] == i or _J[p] == i] for i in range(5)]

# midsection elementwise dtype
MID = FP16


DEBUG = False


def build_bass():
    nc = bacc.Bacc("TRN2", target_bir_lowering=False, debug=False)

    # ---------------- DRAM I/O ----------------
    xt = nc.dram_tensor("xt", [XT_K * P, BC], F32, kind="ExternalInput")
    w1 = {}
    w2 = {}
    b1 = {}
    b2 = {}
    for name, _, K, dh in ENCS:
        w1[name] = nc.dram_tensor(f"w1_{name}", [K * P, dh], F32, kind="ExternalInput")
        w2[name] = nc.dram_tensor(f"w2_{name}", [dh, H], F32, kind="ExternalInput")
        b1[name] = nc.dram_tensor(f"b1_{name}", [P, dh // P], F32, kind="ExternalInput")
        b2[name] = nc.dram_tensor(f"b2_{name}", [P, 4], F32, kind="ExternalInput")
    wg_w = nc.dram_tensor("wg_w", [5 * H, 5], FP16, kind="ExternalInput")
    wg_b = nc.dram_tensor("wg_b", [5, 1], F32, kind="ExternalInput")
    pcat = nc.dram_tensor("pcat", [5, 10], FP16, kind="ExternalInput")
    esel = nc.dram_tensor("esel", [10, 10 * P], FP16, kind="ExternalInput")
    enh_w = nc.dram_tensor("enh_w", [H, H], FP16, kind="ExternalInput")
    enh_b = nc.dram_tensor("enh_b", [P, 4], F32, kind="ExternalInput")
    fus_w = nc.dram_tensor("fus_w", [2 * H, H], FP16, kind="ExternalInput")
    fus_b = nc.dram_tensor("fus_b", [P, 4], F32, kind="ExternalInput")
    out = nc.dram_tensor("out", [H, BC], F32, kind="ExternalOutput")
    dbg = {}
    if DEBUG:
        for nm, shape in [("fps16", [P, 20, BC]), ("stats", [10, BC]),
                          ("ss", [5, BC]), ("wq", [10, BC]), ("fpw", [5, BC]),
                          ("commonT", [P, 4, BC]), ("wsumT", [P, 4, BC]),
                          ("wqrep", [P, 10, BC]), ("mfall", [P, BC])]:
            dt = F32
            dbg[nm] = nc.dram_tensor(f"dbg_{nm}", shape, dt, kind="ExternalOutput")

    with tile.TileContext(nc) as tc:
        kernel_body(
            tc, xt, w1, w2, b1, b2, wg_w, wg_b, pcat, esel, enh_w, enh_b, fus_w, fus_b,
            out, dbg,
        )
    nc.compile()
    return nc


def kernel_body(
    tc, xt, w1, w2, b1, b2, wg_w, wg_b, pcat, esel, enh_w, enh_b, fus_w, fus_b, out,
    dbg={},
):
    nc = tc.nc

    import contextlib

    ctx = contextlib.ExitStack()
    with ctx:
        # -------- pools --------
        persist = ctx.enter_context(tc.tile_pool(name="persist", bufs=1))
        smalls = ctx.enter_context(tc.tile_pool(name="smalls", bufs=1))
        statrows = ctx.enter_context(tc.tile_pool(name="statrows", bufs=1))
        wide_pool = ctx.enter_context(tc.tile_pool(name="widep", bufs=2))
        gs_pool = ctx.enter_context(tc.tile_pool(name="gsp", bufs=1))
        psum_mm = ctx.enter_context(tc.tile_pool(name="psum_mm", bufs=4, space="PSUM"))
        psum_st = ctx.enter_context(tc.tile_pool(name="psum_st", bufs=3, space="PSUM"))
        psum_bc = ctx.enter_context(tc.tile_pool(name="psum_bc", bufs=1, space="PSUM"))
        prod_pool = ctx.enter_context(tc.tile_pool(name="prod", bufs=2))
        t_pool = ctx.enter_context(tc.tile_pool(name="tpool", bufs=2))
        xt_pool = ctx.enter_context(tc.tile_pool(name="xtp", bufs=2))
        w_pool = ctx.enter_context(tc.tile_pool(name="wp", bufs=2))
        h_pool = ctx.enter_context(tc.tile_pool(name="hp", bufs=1))
        gate_pool = ctx.enter_context(tc.tile_pool(name="gatep", bufs=1))

        # -------- persistent tiles --------
        fps16 = persist.tile([P, 20, BC], MID)  # fps.T, ktile = enc*4 + ht
        fps32 = persist.tile([P, 20, BC], F32)  # exact fps.T for d/ss stats
        wqrep = persist.tile([P, 10, BC], MID)
        fpwrep = persist.tile([P, 5, BC], MID)
        mfallrep = persist.tile([P, BC], MID)
        common = persist.tile([P, 4, BC], MID)
        wsum = persist.tile([P, 4, BC], MID)
        enh_sb = persist.tile([P, 4, BC], MID)
        stats = persist.tile([10, BC], F32)  # pair dots d
        ss_t = persist.tile([5, BC], MID)  # squared norms
        l5 = persist.tile([5, BC], MID)
        ones_colf = persist.tile([P, 1], F32)
        ones_col16 = persist.tile([P, 1], MID)
        ones_row16 = persist.tile([1, P], MID)
        pcat_sb = persist.tile([5, 10], MID)
        esel_sb = persist.tile([10, 10 * P], MID)
        biases = {}
        for name, _, K, dh in ENCS:
            biases[name] = (
                persist.tile([P, dh // P], F32, name=f"b1sb_{name}"),
                persist.tile([P, 4], F32, name=f"b2sb_{name}"),
            )
        wgb_sb = persist.tile([5, 1], F32)
        enhb_sb = persist.tile([P, 4], F32)
        fusb_sb = persist.tile([P, 4], F32)

        nc.vector.memset(ones_colf, 1.0)
        nc.vector.memset(ones_col16, 1.0)
        nc.vector.memset(ones_row16, 1.0)
        nc.sync.dma_start(pcat_sb, pcat.ap())
        nc.sync.dma_start(esel_sb, esel.ap())
        for name, _, K, dh in ENCS:
            nc.sync.dma_start(biases[name][0], b1[name].ap())
            nc.sync.dma_start(biases[name][1], b2[name].ap())
        nc.sync.dma_start(wgb_sb, wg_b.ap())
        nc.sync.dma_start(enhb_sb, enh_b.ap())
        nc.sync.dma_start(fusb_sb, fus_b.ap())

        xt_view = xt.ap().rearrange("(ko p) n -> p ko n", p=P)

        # ================= Phase A: encoders (+ interleaved stats) =========
        def stat_row_to(dst, row, ps, nm):
            srow = statrows.tile([1, BC], dst.dtype, tag="statrow", name=f"srow_{nm}")
            nc.scalar.activation(srow, ps, AF.Copy)
            nc.sync.dma_start(dst[row : row + 1, :], srow)

        def emit_d_group(p, engine):
            ps = psum_st.tile([1, BC], F32, tag="stps", name=f"d_{p}")
            for ht in range(4):
                pr = prod_pool.tile([P, BC], F32, tag="prodf")
                engine.tensor_mul(
                    pr, fps32[:, _I[p] * 4 + ht, :], fps32[:, _J[p] * 4 + ht, :]
                )
                nc.tensor.matmul(ps, ones_colf, pr, start=(ht == 0), stop=(ht == 3))
            stat_row_to(stats, p, ps, f"d{p}")

        def emit_ss_group(i):
            ps = psum_st.tile([1, BC], F32, tag="stps", name=f"ss_{i}")
            for ht in range(4):
                sq = prod_pool.tile([P, BC], MID, tag="sq16")
                nc.scalar.square(sq, fps32[:, i * 4 + ht, :])
                nc.tensor.matmul(ps, ones_col16, sq, start=(ht == 0), stop=(ht == 3))
            stat_row_to(ss_t, i, ps, f"ss{i}")

        done_encs = []
        for name in ORDER:
            ei, (_, _, K, dh) = ENC_BY_NAME[name]
            M = dh // P
            b1_sb, b2_sb = biases[name]
            # ---- layer 1: h.T[dh, BC] = relu(w1.T @ x.T + b1) ----
            psums = [
                psum_mm.tile([P, BC], F32, tag="mmps", name=f"l1_{name}_{m}")
                for m in range(M)
            ]
            h_sb = h_pool.tile([P, 4, BC], F32, tag="htile")
            kdone = 0
            for kc0 in range(0, K, 4):
                kn = min(4, K - kc0)
                xt_t = xt_pool.tile([P, 4, BC], F32, tag="xt")
                nc.sync.dma_start(
                    xt_t[:, :kn, :],
                    xt_view[:, XT_OFF[ei] + kc0 : XT_OFF[ei] + kc0 + kn, :],
                )
                w1_t = w_pool.tile([P, 4, 512], F32, tag="w1")
                nc.sync.dma_start(
                    w1_t[:, :kn, :dh],
                    w1[name].ap()[kc0 * P : (kc0 + kn) * P, :].rearrange(
                        "(ko p) m -> p ko m", p=P
                    ),
                )
                for m in range(M):
                    for k in range(kn):
                        nc.tensor.matmul(
                            psums[m],
                            w1_t[:, k, m * P : (m + 1) * P],
                            xt_t[:, k, :],
                            start=(kdone + k == 0),
                            stop=(kdone + k == K - 1),
                        )
                kdone += kn
            for m in range(M):
                nc.scalar.activation(
                    h_sb[:, m, :], psums[m], AF.Relu, bias=b1_sb[:, m : m + 1]
                )
            # ---- layer 2: fps.T[H, BC] = w2.T @ h.T + b2 ----
            w2_t = w_pool.tile([P, 4, 512], F32, tag="w1")
            nc.sync.dma_start(
                w2_t[:, :M, :], w2[name].ap().rearrange("(ko p) m -> p ko m", p=P)
            )
            for m in range(4):
                ps = psum_mm.tile([P, BC], F32, tag="mmps", name=f"l2_{name}_{m}")
                for k in range(M):
                    nc.tensor.matmul(
                        ps,
                        w2_t[:, k, m * P : (m + 1) * P],
                        h_sb[:, k, :],
                        start=(k == 0),
                        stop=(k == M - 1),
                    )
                nc.scalar.activation(
                    fps32[:, ei * 4 + m, :], ps, AF.Identity, bias=b2_sb[:, m : m + 1]
                )
                nc.scalar.activation(
                    fps16[:, ei * 4 + m, :], ps, AF.Identity, bias=b2_sb[:, m : m + 1]
                )
            # ---- interleaved stats for this encoder + completed pairs ----
            emit_ss_group(ei)
            for prev in done_encs:
                pkey = (min(prev, ei), max(prev, ei))
                p = PAIR_IDX[pkey]
                # pairs completed before the last encoder overlap phase A on
                # GpSimd; the final encoder's pairs go to the (then-idle) DVE
                eng = nc.gpsimd if name != ORDER[-1] else nc.vector
                emit_d_group(p, eng)
            done_encs.append(ei)

        # ================= Phase B: softmax over selected pairs ============
        # ln of squared norms, then pairlog[p] = ln(ss_I) + ln(ss_J)
        nc.scalar.activation(l5, ss_t, AF.Ln)
        pl_ps = psum_st.tile([10, BC], F32, tag="stps", name="pl")
        nc.tensor.matmul(pl_ps, pcat_sb, l5, start=True, stop=True)
        invnn = smalls.tile([10, BC], MID)  # 1/(norm_I*norm_J)
        nc.scalar.activation(invnn, pl_ps, AF.Exp, scale=-0.5)
        sims = smalls.tile([10, BC], MID)
        nc.vector.tensor_mul(sims, stats[0:10, :], invnn)
        e0 = smalls.tile([10, BC], MID)
        nc.scalar.activation(e0, sims, AF.Exp)
        e_sb = smalls.tile([10, BC], MID)
        # e = (d > 0) * exp(sims)
        nc.vector.scalar_tensor_tensor(
            e_sb, in0=stats[0:10, :], scalar=0.0, in1=e0, op0=ALU.is_gt, op1=ALU.mult
        )
        den_ps = psum_st.tile([1, BC], F32, tag="stps", name="den")
        nc.tensor.matmul(den_ps, ones_col16[0:10, :], e_sb, start=True, stop=True)
        # mean-fallback weight row: 0.2 * (1 - any(sel))
        mfr = smalls.tile([1, BC], MID)
        nc.vector.tensor_scalar(
            mfr, in0=den_ps, scalar1=0.0, scalar2=-0.2, op0=ALU.is_gt, op1=ALU.mult
        )
        mfr2 = smalls.tile([1, BC], MID)
        nc.vector.tensor_scalar_add(mfr2, mfr, 0.2)
        mfr = mfr2
        # 1/denom on DVE (off the ACT critical path, no table switches);
        # denom is 0 (no sel) or > 1, so clamp at 1
        den_sb = smalls.tile([1, BC], F32)
        nc.vector.tensor_scalar_max(den_sb, den_ps, 1.0)
        recip = smalls.tile([1, BC], MID)
        with nc.allow_low_precision(reason="pair softmax weights tolerate fp16"):
            nc.vector.reciprocal(recip, den_sb)
        rr_ps = psum_st.tile([10, BC], F32, tag="stps", name="rr")
        nc.tensor.matmul(rr_ps, ones_row16[:, 0:10], recip, start=True, stop=True)
        wq_sb = smalls.tile([10, BC], MID)
        # wq = 0.5 * e / denom  (0.5 from the cf definition)
        nc.vector.scalar_tensor_tensor(
            wq_sb, in0=e_sb, scalar=0.5, in1=rr_ps, op0=ALU.mult, op1=ALU.mult
        )

        def broadcast(dst, src_tile, row, nm):
            # out[r, b] = sum_k esel[k, row*128+r] * src[k, b] = src[row, b]
            ksel = src_tile.shape[0]
            bc_ps = psum_bc.tile([P, BC], F32, tag="bcps", name=nm)
            nc.tensor.matmul(
                bc_ps,
                esel_sb[0:ksel, row * P : (row + 1) * P],
                src_tile,
                start=True,
                stop=True,
            )
            nc.scalar.activation(dst, bc_ps, AF.Copy)

        # learned per-fingerprint fusion weights fpw (softmax over 5)
        wg_sb = persist.tile([P, 20, 5], FP16)
        nc.sync.dma_start(wg_sb, wg_w.ap().rearrange("(ko p) m -> p ko m", p=P))
        z_ps = psum_st.tile([5, BC], F32, tag="stps", name="zgate")
        for kt in range(20):
            nc.tensor.matmul(
                z_ps, wg_sb[:, kt, :], fps16[:, kt, :], start=(kt == 0), stop=(kt == 19)
            )
        ez = smalls.tile([5, BC], MID)
        nc.scalar.activation(ez, z_ps, AF.Exp, bias=wgb_sb[0:5, :])
        sez_ps = psum_st.tile([1, BC], F32, tag="stps", name="sez")
        nc.tensor.matmul(sez_ps, ones_col16[0:5, :], ez, start=True, stop=True)
        rez = smalls.tile([1, BC], MID)
        sez_sb = smalls.tile([1, BC], F32, tag="lnrow", name="sez_sb")
        nc.scalar.activation(sez_sb, sez_ps, AF.Copy)
        with nc.allow_low_precision(reason="fusion softmax weights tolerate fp16"):
            nc.vector.reciprocal(rez, sez_sb)
        rz_ps = psum_st.tile([5, BC], F32, tag="stps", name="rz")
        nc.tensor.matmul(rz_ps, ones_row16[:, 0:5], rez, start=True, stop=True)
        fpw_sb = smalls.tile([5, BC], MID)
        nc.vector.tensor_mul(fpw_sb, ez, rz_ps)

        for i in range(5):
            broadcast(fpwrep[:, i, :], fpw_sb, i, f"bc_fpw{i}")
        for p in range(10):
            broadcast(wqrep[:, p, :], wq_sb, p, f"bc_wq{p}")
        broadcast(mfallrep, mfr, 0, "bc_mf")

        # ================= Phase C: masked aggregation =====================
        fps_by_ht = fps16.rearrange("p (i h) n -> p h i n", h=4)
        for ht in range(4):
            # pair products, all 10 in one wide tile
            prodw = wide_pool.tile([P, 10, BC], MID, tag="prodw")
            for p in range(10):
                nc.gpsimd.tensor_mul(
                    prodw[:, p, :],
                    fps16[:, _I[p] * 4 + ht, :],
                    fps16[:, _J[p] * 4 + ht, :],
                )
            # maskw_p = (prod_p > 0) * wq_p, one wide fused op
            maskw = wide_pool.tile([P, 10, BC], MID, tag="prodw", name=f"maskw{ht}")
            nc.vector.scalar_tensor_tensor(
                maskw, in0=prodw, scalar=0.0, in1=wqrep, op0=ALU.is_gt, op1=ALU.mult
            )
            # G_i = sum of the 4 maskw of pairs containing i, + mean-fallback
            # (pure tree, no in-place RMW: in-place DVE adds run ~3x slower)
            gs = gs_pool.tile([P, 5, BC], MID, tag="g")
            for i in range(5):
                pa, pb, pc_, pd = PAIRS_OF[i]
                ga = t_pool.tile([P, BC], MID, tag="gtmp", name=f"ga{ht}_{i}")
                gb = t_pool.tile([P, BC], MID, tag="gtmp2", name=f"gb{ht}_{i}")
                gc = t_pool.tile([P, BC], MID, tag="gtmp3", name=f"gc{ht}_{i}")
                nc.vector.tensor_add(ga, maskw[:, pa, :], maskw[:, pb, :])
                nc.vector.tensor_add(gb, maskw[:, pc_, :], maskw[:, pd, :])
                nc.vector.tensor_add(gc, ga, gb)
                nc.vector.tensor_add(gs[:, i, :], gc, mfallrep)
            # common.T[ht] = sum_i fps_i.T * G_i  (wide mult + pair tree)
            tuw = wide_pool.tile([P, 10, BC], MID, tag="prodw", name=f"tuw{ht}")
            tw = tuw[:, 0:5, :]
            uw = tuw[:, 5:10, :]
            nc.vector.tensor_mul(tw, fps_by_ht[:, ht, :, :], gs)
            r1 = t_pool.tile([P, 2, BC], MID, tag="r1", name=f"r1_{ht}")
            nc.vector.tensor_add(r1, tw[:, 0:4:2, :], tw[:, 1:4:2, :])
            r2 = t_pool.tile([P, BC], MID, tag="gtmp", name=f"r2_{ht}")
            nc.vector.tensor_add(r2, r1[:, 0, :], r1[:, 1, :])
            nc.vector.tensor_add(common[:, ht, :], r2, tw[:, 4, :])
            # wsum.T[ht] likewise with the learned fusion weights
            nc.vector.tensor_mul(uw, fps_by_ht[:, ht, :, :], fpwrep)
            u1 = t_pool.tile([P, 2, BC], MID, tag="r1", name=f"u1_{ht}")
            nc.vector.tensor_add(u1, uw[:, 0:4:2, :], uw[:, 1:4:2, :])
            u2 = t_pool.tile([P, BC], MID, tag="gtmp2", name=f"u2_{ht}")
            nc.vector.tensor_add(u2, u1[:, 0, :], u1[:, 1, :])
            nc.vector.tensor_add(wsum[:, ht, :], u2, uw[:, 4, :])

        if dbg:
            nc.gpsimd.dma_start(dbg["fps16"].ap(), fps16)
            nc.sync.dma_start(dbg["stats"].ap(), stats)
            nc.sync.dma_start(dbg["ss"].ap(), ss_t)
            nc.gpsimd.dma_start(dbg["wq"].ap(), wq_sb)
            nc.gpsimd.dma_start(dbg["fpw"].ap(), fpw_sb)
            nc.gpsimd.dma_start(dbg["commonT"].ap(), common)
            nc.gpsimd.dma_start(dbg["wsumT"].ap(), wsum)
            nc.gpsimd.dma_start(dbg["wqrep"].ap(), wqrep)
            nc.gpsimd.dma_start(dbg["mfall"].ap(), mfallrep)

        # ================= Phase D: enhance + fuse =================
        ew_t = w_pool.tile([P, 4, 512], FP16, tag="w16", name="ew_t")
        nc.sync.dma_start(ew_t, enh_w.ap().rearrange("(ko p) m -> p ko m", p=P))
        for m in range(4):
            ps = psum_mm.tile([P, BC], F32, tag="mmps", name=f"enh_{m}")
            for k in range(4):
                nc.tensor.matmul(
                    ps,
                    ew_t[:, k, m * P : (m + 1) * P],
                    common[:, k, :],
                    start=(k == 0),
                    stop=(k == 3),
                )
            gate = gate_pool.tile([P, BC], MID, tag="gate")
            nc.scalar.activation(gate, ps, AF.Sigmoid, bias=enhb_sb[:, m : m + 1])
            nc.vector.tensor_mul(enh_sb[:, m, :], common[:, m, :], gate)

        fw_view = fus_w.ap().rearrange("(ko p) m -> p ko m", p=P)
        fw_lo = w_pool.tile([P, 4, 512], FP16, tag="w16", name="fw_lo")
        nc.sync.dma_start(fw_lo, fw_view[:, 0:4, :])
        fw_hi = w_pool.tile([P, 4, 512], FP16, tag="w16", name="fw_hi")
        nc.sync.dma_start(fw_hi, fw_view[:, 4:8, :])
        out_view = out.ap().rearrange("(m p) n -> p m n", p=P)
        for m in range(4):
            ps = psum_mm.tile([P, BC], F32, tag="mmps", name=f"fus_{m}")
            for k in range(8):
                rhs = wsum[:, k, :] if k < 4 else enh_sb[:, k - 4, :]
                fw_t = fw_lo if k < 4 else fw_hi
                nc.tensor.matmul(
                    ps,
                    fw_t[:, k % 4, m * P : (m + 1) * P],
                    rhs,
                    start=(k == 0),
                    stop=(k == 7),
                )
            o_sb = gate_pool.tile([P, BC], F32, tag="osb")
            nc.scalar.activation(o_sb, ps, AF.Identity, bias=fusb_sb[:, m : m + 1])
            nc.sync.dma_start(out_view[:, m, :], o_sb)


def prep_inputs(inputs):
    """Host-side: build the per-core in_maps from full inputs."""
    x = np.asarray(inputs["fp_features"], np.float32)

    def pad_rows(a, rows):
        a = np.asarray(a, np.float32)
        if a.shape[0] == rows:
            return a
        out = np.zeros((rows, a.shape[1]), np.float32)
        out[: a.shape[0]] = a
        return out

    # padded transposed x, shared prep then per-core column slices
    xt_full = np.zeros((XT_K * P, B), np.float32)
    offs_in = np.cumsum([0, AP_D, MA_D, MB_D, MC_D])
    for ei, (name, din, K, dh) in enumerate(ENCS):
        seg = x[:, offs_in[ei] : offs_in[ei] + din]  # [B, din]
        xt_full[XT_OFF[ei] * P : XT_OFF[ei] * P + din, :] = np.ascontiguousarray(seg.T)

    common_map = {}
    for ei, (name, din, K, dh) in enumerate(ENCS):
        common_map[f"w1_{name}"] = pad_rows(inputs[f"{name}_w1"], K * P)
        common_map[f"w2_{name}"] = np.asarray(inputs[f"{name}_w2"], np.float32)
        common_map[f"b1_{name}"] = (
            np.asarray(inputs[f"{name}_b1"], np.float32).reshape(dh // P, P).T.copy()
        )
        common_map[f"b2_{name}"] = (
            np.asarray(inputs[f"{name}_b2"], np.float32).reshape(4, P).T.copy()
        )
    common_map["wg_w"] = np.asarray(inputs["wg_w"], np.float32).astype(np.float16)
    common_map["wg_b"] = np.asarray(inputs["wg_b"], np.float32).reshape(5, 1)
    pcat = np.zeros((5, 10), np.float32)
    for p in range(10):
        pcat[_I[p], p] = 1.0
        pcat[_J[p], p] = 1.0
    common_map["pcat"] = pcat.astype(np.float16)
    esel = np.zeros((10, 10 * 128), np.float16)
    for p in range(10):
        esel[p, p * 128 : (p + 1) * 128] = 1.0
    common_map["esel"] = esel
    common_map["enh_w"] = np.asarray(inputs["enh_w"], np.float16)
    common_map["enh_b"] = np.asarray(inputs["enh_b"], np.float32).reshape(4, P).T.copy()
    common_map["fus_w"] = np.asarray(inputs["fus_w"], np.float16)
    common_map["fus_b"] = np.asarray(inputs["fus_b"], np.float32).reshape(4, P).T.copy()

    in_maps = []
    for c in range(N_CORES):
        m = dict(common_map)
        m["xt"] = np.ascontiguousarray(xt_full[:, c * BC : (c + 1) * BC])
        in_maps.append(m)
    return in_maps


_NC_CACHE = None


def kernel(**inputs) -> np.ndarray:
    global _NC_CACHE
    if _NC_CACHE is None:
        _NC_CACHE = build_bass()
    nc = _NC_CACHE
    in_maps = prep_inputs(inputs)
    res = run_bass_kernel_spmd(nc, in_maps, core_ids=list(range(N_CORES)))
    outs = [res.results[c]["out"] for c in range(N_CORES)]  # each [H, BC]
    full = np.concatenate([o.T for o in outs], axis=0)  # [B, H]
    return np.ascontiguousarray(full.astype(np.float32))





# revision 11
# speedup vs baseline: 1.7351x; 1.7351x over previous
"""Trainium2 Bass kernel for nn_CommonFeatureExtractor.

Data-parallel over 8 NeuronCores: batch dim (4096) sharded into 8 x 512,
weights replicated. Inside each core everything is computed in the
"transposed" layout [feature_on_partitions, batch_free] so that all matmul
contractions (which run over the partition axis on the PE) need no on-chip
transposes: the host feeds x already transposed and the weights are natural
[din, dout] = [K, M] layout, which is exactly what the PE's lhsT wants.

Pipeline per core (B=512 samples):
  A) 5 encoder MLPs (fp32 data, fp32r matmuls), fps.T stored bf16 [128,20,512]
  B) stats: pair products/squares (bf16) -> PE ones-matmul partition
     reductions -> d[10,B], ss[5,B]; softmax over selected pairs via
     ln/exp trick; per-pair weights broadcast to [128,B] via K=1 matmuls
  C) masked aggregation: G_i = sum_{pairs p containing i} (prod_p>0)*wq_p
     (+ mean-fallback), common.T = sum_i fps_i.T * G_i; wsum.T likewise with
     learned softmax gate weights
  D) enhance (sigmoid gate) + fuse matmuls -> fused.T [512, 512] -> host
     transposes back and concatenates.
"""

import numpy as np

import concourse.bass as bass
import concourse.mybir as mybir
import concourse.tile as tile
from concourse import bacc
from concourse.bass_utils import run_bass_kernel_spmd

F32 = mybir.dt.float32
F32R = mybir.dt.float32r
BF16 = mybir.dt.bfloat16
FP16 = mybir.dt.float16
ALU = mybir.AluOpType
AF = mybir.ActivationFunctionType

N_CORES = 8
B = 4096
BC = B // N_CORES  # 512 samples per core
H = 512
P = 128

AP_D, MA_D, MB_D, MC_D, PH_D = 2048, 167, 2048, 2048, 27
# encoders: (name, din, padded K tiles, hidden dh, M tiles = dh/128)
ENCS = [
    ("ap", AP_D, 16, 512),
    ("ma", MA_D, 2, 256),
    ("mb", MB_D, 16, 512),
    ("mc", MC_D, 16, 512),
    ("ph", PH_D, 1, 128),
]
XT_K = sum(e[2] for e in ENCS)  # 51 padded k-tiles of x
XT_OFF = np.cumsum([0] + [e[2] for e in ENCS])[:-1]  # [0,16,18,34,50]

_I = [0, 0, 0, 0, 1, 1, 1, 2, 2, 3]
_J = [1, 2, 3, 4, 2, 3, 4, 3, 4, 4]
PAIR_IDX = {(_I[p], _J[p]): p for p in range(10)}
# compute order: small encoders first so most pair-stats overlap phase A
ORDER = ["ma", "ph", "ap", "mb", "mc"]
ENC_BY_NAME = {e[0]: (i, e) for i, e in enumerate(ENCS)}
# pairs containing encoder i
PAIRS_OF = [[p for p in range(10) if _I[p] == i or _J[p] == i] for i in range(5)]

# midsection elementwise dtype
MID = FP16


DEBUG = False


def build_bass():
    nc = bacc.Bacc("TRN2", target_bir_lowering=False, debug=False)

    # ---------------- DRAM I/O ----------------
    xt = nc.dram_tensor("xt", [XT_K * P, BC], FP16, kind="ExternalInput")
    w1 = {}
    w2 = {}
    b1 = {}
    b2 = {}
    for name, _, K, dh in ENCS:
        w1[name] = nc.dram_tensor(f"w1_{name}", [K * P, dh], FP16, kind="ExternalInput")
        w2[name] = nc.dram_tensor(f"w2_{name}", [dh, H], FP16, kind="ExternalInput")
        b1[name] = nc.dram_tensor(f"b1_{name}", [P, dh // P], F32, kind="ExternalInput")
        b2[name] = nc.dram_tensor(f"b2_{name}", [P, 4], F32, kind="ExternalInput")
    wg_w = nc.dram_tensor("wg_w", [5 * H, 5], FP16, kind="ExternalInput")
    wg_b = nc.dram_tensor("wg_b", [5, 1], F32, kind="ExternalInput")
    pcat = nc.dram_tensor("pcat", [5, 10], FP16, kind="ExternalInput")
    esel = nc.dram_tensor("esel", [10, 10 * P], FP16, kind="ExternalInput")
    enh_w = nc.dram_tensor("enh_w", [H, H], FP16, kind="ExternalInput")
    enh_b = nc.dram_tensor("enh_b", [P, 4], F32, kind="ExternalInput")
    fus_w = nc.dram_tensor("fus_w", [2 * H, H], FP16, kind="ExternalInput")
    fus_b = nc.dram_tensor("fus_b", [P, 4], F32, kind="ExternalInput")
    out = nc.dram_tensor("out", [H, BC], F32, kind="ExternalOutput")
    dbg = {}
    if DEBUG:
        for nm, shape in [("fps16", [P, 20, BC]), ("stats", [10, BC]),
                          ("ss", [5, BC]), ("wq", [10, BC]), ("fpw", [5, BC]),
                          ("commonT", [P, 4, BC]), ("wsumT", [P, 4, BC]),
                          ("wqrep", [P, 10, BC]), ("mfall", [P, BC])]:
            dt = F32
            dbg[nm] = nc.dram_tensor(f"dbg_{nm}", shape, dt, kind="ExternalOutput")

    with tile.TileContext(nc) as tc:
        kernel_body(
            tc, xt, w1, w2, b1, b2, wg_w, wg_b, pcat, esel, enh_w, enh_b, fus_w, fus_b,
            out, dbg,
        )
    nc.compile()
    return nc


def kernel_body(
    tc, xt, w1, w2, b1, b2, wg_w, wg_b, pcat, esel, enh_w, enh_b, fus_w, fus_b, out,
    dbg={},
):
    nc = tc.nc

    import contextlib

    ctx = contextlib.ExitStack()
    with ctx:
        # -------- pools --------
        persist = ctx.enter_context(tc.tile_pool(name="persist", bufs=1))
        smalls = ctx.enter_context(tc.tile_pool(name="smalls", bufs=1))
        statrows = ctx.enter_context(tc.tile_pool(name="statrows", bufs=1))
        wide_pool = ctx.enter_context(tc.tile_pool(name="widep", bufs=2))
        gs_pool = ctx.enter_context(tc.tile_pool(name="gsp", bufs=1))
        psum_mm = ctx.enter_context(tc.tile_pool(name="psum_mm", bufs=4, space="PSUM"))
        psum_st = ctx.enter_context(tc.tile_pool(name="psum_st", bufs=3, space="PSUM"))
        psum_bc = ctx.enter_context(tc.tile_pool(name="psum_bc", bufs=1, space="PSUM"))
        prod_pool = ctx.enter_context(tc.tile_pool(name="prod", bufs=2))
        t_pool = ctx.enter_context(tc.tile_pool(name="tpool", bufs=2))
        xt_pool = ctx.enter_context(tc.tile_pool(name="xtp", bufs=2))
        w_pool = ctx.enter_context(tc.tile_pool(name="wp", bufs=2))
        h_pool = ctx.enter_context(tc.tile_pool(name="hp", bufs=1))
        gate_pool = ctx.enter_context(tc.tile_pool(name="gatep", bufs=1))

        # -------- persistent tiles --------
        fps16 = persist.tile([P, 20, BC], MID)  # fps.T, ktile = enc*4 + ht
        wqrep = persist.tile([P, 10, BC], MID)
        fpwrep = persist.tile([P, 5, BC], MID)
        mfallrep = persist.tile([P, BC], MID)
        common = persist.tile([P, 4, BC], MID)
        wsum = persist.tile([P, 4, BC], MID)
        enh_sb = persist.tile([P, 4, BC], MID)
        stats = persist.tile([10, BC], F32)  # pair dots d
        ss_t = persist.tile([5, BC], MID)  # squared norms
        l5 = persist.tile([5, BC], MID)
        ones_col16 = persist.tile([P, 1], MID)
        ones_row16 = persist.tile([1, P], MID)
        pcat_sb = persist.tile([5, 10], MID)
        esel_sb = persist.tile([10, 10 * P], MID)
        biases = {}
        for name, _, K, dh in ENCS:
            biases[name] = (
                persist.tile([P, dh // P], F32, name=f"b1sb_{name}"),
                persist.tile([P, 4], F32, name=f"b2sb_{name}"),
            )
        wgb_sb = persist.tile([5, 1], F32)
        enhb_sb = persist.tile([P, 4], F32)
        fusb_sb = persist.tile([P, 4], F32)

        nc.vector.memset(ones_col16, 1.0)
        nc.vector.memset(ones_row16, 1.0)
        nc.sync.dma_start(pcat_sb, pcat.ap())
        nc.sync.dma_start(esel_sb, esel.ap())
        for name, _, K, dh in ENCS:
            nc.sync.dma_start(biases[name][0], b1[name].ap())
            nc.sync.dma_start(biases[name][1], b2[name].ap())
        nc.sync.dma_start(wgb_sb, wg_b.ap())
        nc.sync.dma_start(enhb_sb, enh_b.ap())
        nc.sync.dma_start(fusb_sb, fus_b.ap())

        xt_view = xt.ap().rearrange("(ko p) n -> p ko n", p=P)

        # ================= Phase A: encoders (+ interleaved stats) =========
        def stat_row_to(dst, row, ps, nm):
            srow = statrows.tile([1, BC], dst.dtype, tag="statrow", name=f"srow_{nm}")
            nc.scalar.activation(srow, ps, AF.Copy)
            nc.sync.dma_start(dst[row : row + 1, :], srow)

        def emit_d_group(p, engine):
            ps = psum_st.tile([1, BC], F32, tag="stps", name=f"d_{p}")
            for ht in range(4):
                pr = prod_pool.tile([P, BC], MID, tag="prodf")
                engine.tensor_mul(
                    pr, fps16[:, _I[p] * 4 + ht, :], fps16[:, _J[p] * 4 + ht, :]
                )
                nc.tensor.matmul(ps, ones_col16, pr, start=(ht == 0), stop=(ht == 3))
            stat_row_to(stats, p, ps, f"d{p}")

        def emit_ss_group(i):
            ps = psum_st.tile([1, BC], F32, tag="stps", name=f"ss_{i}")
            for ht in range(4):
                sq = prod_pool.tile([P, BC], MID, tag="sq16")
                nc.scalar.square(sq, fps16[:, i * 4 + ht, :])
                nc.tensor.matmul(ps, ones_col16, sq, start=(ht == 0), stop=(ht == 3))
            stat_row_to(ss_t, i, ps, f"ss{i}")

        done_encs = []
        for name in ORDER:
            ei, (_, _, K, dh) = ENC_BY_NAME[name]
            M = dh // P
            b1_sb, b2_sb = biases[name]
            # ---- layer 1: h.T[dh, BC] = relu(w1.T @ x.T + b1) ----
            psums = [
                psum_mm.tile([P, BC], F32, tag="mmps", name=f"l1_{name}_{m}")
                for m in range(M)
            ]
            h_sb = h_pool.tile([P, 4, BC], MID, tag="htile")
            kdone = 0
            for kc0 in range(0, K, 4):
                kn = min(4, K - kc0)
                xt_t = xt_pool.tile([P, 4, BC], FP16, tag="xt")
                nc.sync.dma_start(
                    xt_t[:, :kn, :],
                    xt_view[:, XT_OFF[ei] + kc0 : XT_OFF[ei] + kc0 + kn, :],
                )
                w1_t = w_pool.tile([P, 4, 512], FP16, tag="w1")
                nc.sync.dma_start(
                    w1_t[:, :kn, :dh],
                    w1[name].ap()[kc0 * P : (kc0 + kn) * P, :].rearrange(
                        "(ko p) m -> p ko m", p=P
                    ),
                )
                for m in range(M):
                    for k in range(kn):
                        nc.tensor.matmul(
                            psums[m],
                            w1_t[:, k, m * P : (m + 1) * P],
                            xt_t[:, k, :],
                            start=(kdone + k == 0),
                            stop=(kdone + k == K - 1),
                        )
                kdone += kn
            for m in range(M):
                nc.scalar.activation(
                    h_sb[:, m, :], psums[m], AF.Relu, bias=b1_sb[:, m : m + 1]
                )
            # ---- layer 2: fps.T[H, BC] = w2.T @ h.T + b2 ----
            w2_t = w_pool.tile([P, 4, 512], FP16, tag="w1")
            nc.sync.dma_start(
                w2_t[:, :M, :], w2[name].ap().rearrange("(ko p) m -> p ko m", p=P)
            )
            for m in range(4):
                ps = psum_mm.tile([P, BC], F32, tag="mmps", name=f"l2_{name}_{m}")
                for k in range(M):
                    nc.tensor.matmul(
                        ps,
                        w2_t[:, k, m * P : (m + 1) * P],
                        h_sb[:, k, :],
                        start=(k == 0),
                        stop=(k == M - 1),
                    )
                nc.scalar.activation(
                    fps16[:, ei * 4 + m, :], ps, AF.Identity, bias=b2_sb[:, m : m + 1]
                )
            # ---- interleaved stats for this encoder + completed pairs ----
            emit_ss_group(ei)
            for prev in done_encs:
                pkey = (min(prev, ei), max(prev, ei))
                p = PAIR_IDX[pkey]
                # fp16 products are cheap on DVE, which is otherwise idle in
                # phase A (phase C's wide ops keep Pool busy later instead)
                emit_d_group(p, nc.vector)
            done_encs.append(ei)

        # ================= Phase B: softmax over selected pairs ============
        # ln of squared norms, then pairlog[p] = ln(ss_I) + ln(ss_J)
        nc.scalar.activation(l5, ss_t, AF.Ln)
        pl_ps = psum_st.tile([10, BC], F32, tag="stps", name="pl")
        nc.tensor.matmul(pl_ps, pcat_sb, l5, start=True, stop=True)
        invnn = smalls.tile([10, BC], MID)  # 1/(norm_I*norm_J)
        nc.scalar.activation(invnn, pl_ps, AF.Exp, scale=-0.5)
        sims = smalls.tile([10, BC], MID)
        nc.vector.tensor_mul(sims, stats[0:10, :], invnn)
        e0 = smalls.tile([10, BC], MID)
        nc.scalar.activation(e0, sims, AF.Exp)
        e_sb = smalls.tile([10, BC], MID)
        # e = (d > 0) * exp(sims)
        nc.vector.scalar_tensor_tensor(
            e_sb, in0=stats[0:10, :], scalar=0.0, in1=e0, op0=ALU.is_gt, op1=ALU.mult
        )
        den_ps = psum_st.tile([1, BC], F32, tag="stps", name="den")
        nc.tensor.matmul(den_ps, ones_col16[0:10, :], e_sb, start=True, stop=True)
        # mean-fallback weight row: 0.2 * (1 - any(sel))
        mfr = smalls.tile([1, BC], MID)
        nc.vector.tensor_scalar(
            mfr, in0=den_ps, scalar1=0.0, scalar2=-0.2, op0=ALU.is_gt, op1=ALU.mult
        )
        mfr2 = smalls.tile([1, BC], MID)
        nc.vector.tensor_scalar_add(mfr2, mfr, 0.2)
        mfr = mfr2
        # 1/denom on DVE (off the ACT critical path, no table switches);
        # denom is 0 (no sel) or > 1, so clamp at 1
        den_sb = smalls.tile([1, BC], F32)
        nc.vector.tensor_scalar_max(den_sb, den_ps, 1.0)
        recip = smalls.tile([1, BC], MID)
        with nc.allow_low_precision(reason="pair softmax weights tolerate fp16"):
            nc.vector.reciprocal(recip, den_sb)
        rr_ps = psum_st.tile([10, BC], F32, tag="stps", name="rr")
        nc.tensor.matmul(rr_ps, ones_row16[:, 0:10], recip, start=True, stop=True)
        wq_sb = smalls.tile([10, BC], MID)
        # wq = 0.5 * e / denom  (0.5 from the cf definition)
        nc.vector.scalar_tensor_tensor(
            wq_sb, in0=e_sb, scalar=0.5, in1=rr_ps, op0=ALU.mult, op1=ALU.mult
        )

        def broadcast(dst, src_tile, row, nm):
            # out[r, b] = sum_k esel[k, row*128+r] * src[k, b] = src[row, b]
            ksel = src_tile.shape[0]
            bc_ps = psum_bc.tile([P, BC], F32, tag="bcps", name=nm)
            nc.tensor.matmul(
                bc_ps,
                esel_sb[0:ksel, row * P : (row + 1) * P],
                src_tile,
                start=True,
                stop=True,
            )
            nc.scalar.activation(dst, bc_ps, AF.Copy)

        # learned per-fingerprint fusion weights fpw (softmax over 5)
        wg_sb = persist.tile([P, 20, 5], FP16)
        nc.sync.dma_start(wg_sb, wg_w.ap().rearrange("(ko p) m -> p ko m", p=P))
        z_ps = psum_st.tile([5, BC], F32, tag="stps", name="zgate")
        for kt in range(20):
            nc.tensor.matmul(
                z_ps, wg_sb[:, kt, :], fps16[:, kt, :], start=(kt == 0), stop=(kt == 19)
            )
        ez = smalls.tile([5, BC], MID)
        nc.scalar.activation(ez, z_ps, AF.Exp, bias=wgb_sb[0:5, :])
        sez_ps = psum_st.tile([1, BC], F32, tag="stps", name="sez")
        nc.tensor.matmul(sez_ps, ones_col16[0:5, :], ez, start=True, stop=True)
        rez = smalls.tile([1, BC], MID)
        sez_sb = smalls.tile([1, BC], F32, tag="lnrow", name="sez_sb")
        nc.scalar.activation(sez_sb, sez_ps, AF.Copy)
        with nc.allow_low_precision(reason="fusion softmax weights tolerate fp16"):
            nc.vector.reciprocal(rez, sez_sb)
        rz_ps = psum_st.tile([5, BC], F32, tag="stps", name="rz")
        nc.tensor.matmul(rz_ps, ones_row16[:, 0:5], rez, start=True, stop=True)
        fpw_sb = smalls.tile([5, BC], MID)
        nc.vector.tensor_mul(fpw_sb, ez, rz_ps)

        for i in range(5):
            broadcast(fpwrep[:, i, :], fpw_sb, i, f"bc_fpw{i}")
        for p in range(10):
            broadcast(wqrep[:, p, :], wq_sb, p, f"bc_wq{p}")
        broadcast(mfallrep, mfr, 0, "bc_mf")

        # ================= Phase C: masked aggregation =====================
        fps_by_ht = fps16.rearrange("p (i h) n -> p h i n", h=4)
        for ht in range(4):
            # pair products, all 10 in one wide tile
            prodw = wide_pool.tile([P, 10, BC], MID, tag="prodw")
            for p in range(10):
                nc.gpsimd.tensor_mul(
                    prodw[:, p, :],
                    fps16[:, _I[p] * 4 + ht, :],
                    fps16[:, _J[p] * 4 + ht, :],
                )
            # maskw_p = (prod_p > 0) * wq_p, one wide fused op
            maskw = wide_pool.tile([P, 10, BC], MID, tag="prodw", name=f"maskw{ht}")
            nc.vector.scalar_tensor_tensor(
                maskw, in0=prodw, scalar=0.0, in1=wqrep, op0=ALU.is_gt, op1=ALU.mult
            )
            # G_i = sum of the 4 maskw of pairs containing i, + mean-fallback
            # (pure tree, no in-place RMW: in-place DVE adds run ~3x slower)
            gs = gs_pool.tile([P, 5, BC], MID, tag="g")
            for i in range(5):
                pa, pb, pc_, pd = PAIRS_OF[i]
                ga = t_pool.tile([P, BC], MID, tag="gtmp", name=f"ga{ht}_{i}")
                gb = t_pool.tile([P, BC], MID, tag="gtmp2", name=f"gb{ht}_{i}")
                gc = t_pool.tile([P, BC], MID, tag="gtmp3", name=f"gc{ht}_{i}")
                nc.vector.tensor_add(ga, maskw[:, pa, :], maskw[:, pb, :])
                nc.vector.tensor_add(gb, maskw[:, pc_, :], maskw[:, pd, :])
                nc.vector.tensor_add(gc, ga, gb)
                nc.vector.tensor_add(gs[:, i, :], gc, mfallrep)
            # common.T[ht] = sum_i fps_i.T * G_i  (wide mult + pair tree)
            tuw = wide_pool.tile([P, 10, BC], MID, tag="prodw", name=f"tuw{ht}")
            tw = tuw[:, 0:5, :]
            uw = tuw[:, 5:10, :]
            nc.vector.tensor_mul(tw, fps_by_ht[:, ht, :, :], gs)
            r1 = t_pool.tile([P, 2, BC], MID, tag="r1", name=f"r1_{ht}")
            nc.vector.tensor_add(r1, tw[:, 0:4:2, :], tw[:, 1:4:2, :])
            r2 = t_pool.tile([P, BC], MID, tag="gtmp", name=f"r2_{ht}")
            nc.vector.tensor_add(r2, r1[:, 0, :], r1[:, 1, :])
            nc.vector.tensor_add(common[:, ht, :], r2, tw[:, 4, :])
            # wsum.T[ht] likewise with the learned fusion weights
            nc.vector.tensor_mul(uw, fps_by_ht[:, ht, :, :], fpwrep)
            u1 = t_pool.tile([P, 2, BC], MID, tag="r1", name=f"u1_{ht}")
            nc.vector.tensor_add(u1, uw[:, 0:4:2, :], uw[:, 1:4:2, :])
            u2 = t_pool.tile([P, BC], MID, tag="gtmp2", name=f"u2_{ht}")
            nc.vector.tensor_add(u2, u1[:, 0, :], u1[:, 1, :])
            nc.vector.tensor_add(wsum[:, ht, :], u2, uw[:, 4, :])

        if dbg:
            nc.gpsimd.dma_start(dbg["fps16"].ap(), fps16)
            nc.sync.dma_start(dbg["stats"].ap(), stats)
            nc.sync.dma_start(dbg["ss"].ap(), ss_t)
            nc.gpsimd.dma_start(dbg["wq"].ap(), wq_sb)
            nc.gpsimd.dma_start(dbg["fpw"].ap(), fpw_sb)
            nc.gpsimd.dma_start(dbg["commonT"].ap(), common)
            nc.gpsimd.dma_start(dbg["wsumT"].ap(), wsum)
            nc.gpsimd.dma_start(dbg["wqrep"].ap(), wqrep)
            nc.gpsimd.dma_start(dbg["mfall"].ap(), mfallrep)

        # ================= Phase D: enhance + fuse =================
        ew_t = w_pool.tile([P, 4, 512], FP16, tag="w16", name="ew_t")
        nc.sync.dma_start(ew_t, enh_w.ap().rearrange("(ko p) m -> p ko m", p=P))
        for m in range(4):
            ps = psum_mm.tile([P, BC], F32, tag="mmps", name=f"enh_{m}")
            for k in range(4):
                nc.tensor.matmul(
                    ps,
                    ew_t[:, k, m * P : (m + 1) * P],
                    common[:, k, :],
                    start=(k == 0),
                    stop=(k == 3),
                )
            gate = gate_pool.tile([P, BC], MID, tag="gate")
            nc.scalar.activation(gate, ps, AF.Sigmoid, bias=enhb_sb[:, m : m + 1])
            nc.vector.tensor_mul(enh_sb[:, m, :], common[:, m, :], gate)

        fw_view = fus_w.ap().rearrange("(ko p) m -> p ko m", p=P)
        fw_lo = w_pool.tile([P, 4, 512], FP16, tag="w16", name="fw_lo")
        nc.sync.dma_start(fw_lo, fw_view[:, 0:4, :])
        fw_hi = w_pool.tile([P, 4, 512], FP16, tag="w16", name="fw_hi")
        nc.sync.dma_start(fw_hi, fw_view[:, 4:8, :])
        out_view = out.ap().rearrange("(m p) n -> p m n", p=P)
        for m in range(4):
            ps = psum_mm.tile([P, BC], F32, tag="mmps", name=f"fus_{m}")
            for k in range(8):
                rhs = wsum[:, k, :] if k < 4 else enh_sb[:, k - 4, :]
                fw_t = fw_lo if k < 4 else fw_hi
                nc.tensor.matmul(
                    ps,
                    fw_t[:, k % 4, m * P : (m + 1) * P],
                    rhs,
                    start=(k == 0),
                    stop=(k == 7),
                )
            o_sb = gate_pool.tile([P, BC], F32, tag="osb")
            nc.scalar.activation(o_sb, ps, AF.Identity, bias=fusb_sb[:, m : m + 1])
            nc.sync.dma_start(out_view[:, m, :], o_sb)


def prep_inputs(inputs):
    """Host-side: build the per-core in_maps from full inputs."""
    x = np.asarray(inputs["fp_features"], np.float32)

    def pad_rows(a, rows):
        a = np.asarray(a, np.float32)
        if a.shape[0] == rows:
            return a
        out = np.zeros((rows, a.shape[1]), np.float32)
        out[: a.shape[0]] = a
        return out

    # padded transposed x, shared prep then per-core column slices
    xt_full = np.zeros((XT_K * P, B), np.float16)
    offs_in = np.cumsum([0, AP_D, MA_D, MB_D, MC_D])
    for ei, (name, din, K, dh) in enumerate(ENCS):
        seg = x[:, offs_in[ei] : offs_in[ei] + din]  # [B, din]
        xt_full[XT_OFF[ei] * P : XT_OFF[ei] * P + din, :] = seg.T.astype(np.float16)

    common_map = {}
    for ei, (name, din, K, dh) in enumerate(ENCS):
        common_map[f"w1_{name}"] = pad_rows(inputs[f"{name}_w1"], K * P).astype(
            np.float16
        )
        common_map[f"w2_{name}"] = np.asarray(inputs[f"{name}_w2"], np.float16)
        common_map[f"b1_{name}"] = (
            np.asarray(inputs[f"{name}_b1"], np.float32).reshape(dh // P, P).T.copy()
        )
        common_map[f"b2_{name}"] = (
            np.asarray(inputs[f"{name}_b2"], np.float32).reshape(4, P).T.copy()
        )
    common_map["wg_w"] = np.asarray(inputs["wg_w"], np.float32).astype(np.float16)
    common_map["wg_b"] = np.asarray(inputs["wg_b"], np.float32).reshape(5, 1)
    pcat = np.zeros((5, 10), np.float32)
    for p in range(10):
        pcat[_I[p], p] = 1.0
        pcat[_J[p], p] = 1.0
    common_map["pcat"] = pcat.astype(np.float16)
    esel = np.zeros((10, 10 * 128), np.float16)
    for p in range(10):
        esel[p, p * 128 : (p + 1) * 128] = 1.0
    common_map["esel"] = esel
    common_map["enh_w"] = np.asarray(inputs["enh_w"], np.float16)
    common_map["enh_b"] = np.asarray(inputs["enh_b"], np.float32).reshape(4, P).T.copy()
    common_map["fus_w"] = np.asarray(inputs["fus_w"], np.float16)
    common_map["fus_b"] = np.asarray(inputs["fus_b"], np.float32).reshape(4, P).T.copy()

    in_maps = []
    for c in range(N_CORES):
        m = dict(common_map)
        m["xt"] = np.ascontiguousarray(xt_full[:, c * BC : (c + 1) * BC])
        in_maps.append(m)
    return in_maps


_NC_CACHE = None


def kernel(**inputs) -> np.ndarray:
    global _NC_CACHE
    if _NC_CACHE is None:
        _NC_CACHE = build_bass()
    nc = _NC_CACHE
    in_maps = prep_inputs(inputs)
    res = run_bass_kernel_spmd(nc, in_maps, core_ids=list(range(N_CORES)))
    outs = [res.results[c]["out"] for c in range(N_CORES)]  # each [H, BC]
    full = np.concatenate([o.T for o in outs], axis=0)  # [B, H]
    return np.ascontiguousarray(full.astype(np.float32))





# revision 29
# speedup vs baseline: 2.1666x; 1.2487x over previous
"""Trainium2 Bass kernel for nn_CommonFeatureExtractor.

Data-parallel over 8 NeuronCores: batch dim (4096) sharded into 8 x 512,
weights replicated. Inside each core everything is computed in the
"transposed" layout [feature_on_partitions, batch_free] so that all matmul
contractions (which run over the partition axis on the PE) need no on-chip
transposes: the host feeds x already transposed and the weights are natural
[din, dout] = [K, M] layout, which is exactly what the PE's lhsT wants.

Pipeline per core (B=512 samples):
  A) 5 encoder MLPs (fp32 data, fp32r matmuls), fps.T stored bf16 [128,20,512]
  B) stats: pair products/squares (bf16) -> PE ones-matmul partition
     reductions -> d[10,B], ss[5,B]; softmax over selected pairs via
     ln/exp trick; per-pair weights broadcast to [128,B] via K=1 matmuls
  C) masked aggregation: G_i = sum_{pairs p containing i} (prod_p>0)*wq_p
     (+ mean-fallback), common.T = sum_i fps_i.T * G_i; wsum.T likewise with
     learned softmax gate weights
  D) enhance (sigmoid gate) + fuse matmuls -> fused.T [512, 512] -> host
     transposes back and concatenates.
"""

import numpy as np

import concourse.bass as bass
import concourse.mybir as mybir
import concourse.tile as tile
from concourse import bacc
from concourse.bass_utils import run_bass_kernel_spmd

F32 = mybir.dt.float32
F32R = mybir.dt.float32r
BF16 = mybir.dt.bfloat16
FP16 = mybir.dt.float16
ALU = mybir.AluOpType
AF = mybir.ActivationFunctionType

N_CORES = 8
B = 4096
BC = B // N_CORES  # 512 samples per core
H = 512
P = 128

AP_D, MA_D, MB_D, MC_D, PH_D = 2048, 167, 2048, 2048, 27
# encoders: (name, din, padded K tiles, hidden dh, M tiles = dh/128)
ENCS = [
    ("ap", AP_D, 16, 512),
    ("ma", MA_D, 2, 256),
    ("mb", MB_D, 16, 512),
    ("mc", MC_D, 16, 512),
    ("ph", PH_D, 1, 128),
]
XT_K = sum(e[2] for e in ENCS)  # 51 padded k-tiles of x
XT_OFF = np.cumsum([0] + [e[2] for e in ENCS])[:-1]  # [0,16,18,34,50]

_I = [0, 0, 0, 0, 1, 1, 1, 2, 2, 3]
_J = [1, 2, 3, 4, 2, 3, 4, 3, 4, 4]
PAIR_IDX = {(_I[p], _J[p]): p for p in range(10)}
# compute order: small encoders first so most pair-stats overlap phase A
ORDER = ["ma", "ph", "ap", "mb", "mc"]
ENC_BY_NAME = {e[0]: (i, e) for i, e in enumerate(ENCS)}
# pairs containing encoder i
PAIRS_OF = [[p for p in range(10) if _I[p] == i or _J[p] == i] for i in range(5)]

# midsection elementwise dtype
MID = FP16


DEBUG = False


def build_bass():
    nc = bacc.Bacc("TRN2", target_bir_lowering=False, debug=False)

    # ---------------- DRAM I/O ----------------
    xt = nc.dram_tensor("xt", [XT_K * P, BC], FP16, kind="ExternalInput")
    w1 = {}
    w2 = {}
    b1 = {}
    b2 = {}
    for name, _, K, dh in ENCS:
        w1[name] = nc.dram_tensor(f"w1_{name}", [K * P, dh], FP16, kind="ExternalInput")
        w2[name] = nc.dram_tensor(f"w2_{name}", [dh, H], FP16, kind="ExternalInput")
        b1[name] = nc.dram_tensor(f"b1_{name}", [P, dh // P], F32, kind="ExternalInput")
        b2[name] = nc.dram_tensor(f"b2_{name}", [P, 4], F32, kind="ExternalInput")
    wg_w = nc.dram_tensor("wg_w", [5 * H, 5], FP16, kind="ExternalInput")
    wg_b = nc.dram_tensor("wg_b", [5, 1], F32, kind="ExternalInput")
    pcat = nc.dram_tensor("pcat", [5, 10], FP16, kind="ExternalInput")
    esel = nc.dram_tensor("esel", [10, 10 * P], FP16, kind="ExternalInput")
    # one-hot column selectors: dsel[:, g*15:(g+1)*15] has column g all-ones,
    # so matmul(lhsT=dsel_g, rhs) reduces rhs over partitions into row g
    dsel = nc.dram_tensor("dsel", [P, 15 * 15], FP16, kind="ExternalInput")
    enh_w = nc.dram_tensor("enh_w", [H, H], FP16, kind="ExternalInput")
    enh_b = nc.dram_tensor("enh_b", [P, 4], F32, kind="ExternalInput")
    fus_w = nc.dram_tensor("fus_w", [2 * H, H], FP16, kind="ExternalInput")
    fus_b = nc.dram_tensor("fus_b", [P, 4], F32, kind="ExternalInput")
    out = nc.dram_tensor("out", [H, BC], F32, kind="ExternalOutput")
    dbg = {}
    if DEBUG:
        for nm, shape in [("fps16", [P, 20, BC]), ("stats", [10, BC]),
                          ("ss", [5, BC]), ("wq", [10, BC]), ("fpw", [5, BC]),
                          ("commonT", [P, 4, BC]), ("wsumT", [P, 4, BC]),
                          ("wqrep", [P, 10, BC]), ("mfall", [P, BC])]:
            dt = F32
            dbg[nm] = nc.dram_tensor(f"dbg_{nm}", shape, dt, kind="ExternalOutput")

    with tile.TileContext(nc) as tc:
        kernel_body(
            tc, xt, w1, w2, b1, b2, wg_w, wg_b, pcat, esel, dsel, enh_w, enh_b,
            fus_w, fus_b, out, dbg,
        )
    nc.compile()
    return nc


def kernel_body(
    tc, xt, w1, w2, b1, b2, wg_w, wg_b, pcat, esel, dsel, enh_w, enh_b, fus_w, fus_b,
    out, dbg={},
):
    nc = tc.nc

    import contextlib

    ctx = contextlib.ExitStack()
    with ctx:
        # -------- pools --------
        persist = ctx.enter_context(tc.tile_pool(name="persist", bufs=1))
        smalls = ctx.enter_context(tc.tile_pool(name="smalls", bufs=1))
        statrows = ctx.enter_context(tc.tile_pool(name="statrows", bufs=1))
        wide_pool = ctx.enter_context(tc.tile_pool(name="widep", bufs=2))
        gs_pool = ctx.enter_context(tc.tile_pool(name="gsp", bufs=1))
        psum_mm = ctx.enter_context(tc.tile_pool(name="psum_mm", bufs=4, space="PSUM"))
        psum_acc = ctx.enter_context(
            tc.tile_pool(name="psum_acc", bufs=1, space="PSUM")
        )
        psum_acc2 = ctx.enter_context(
            tc.tile_pool(name="psum_acc2", bufs=1, space="PSUM")
        )
        psum_st = ctx.enter_context(tc.tile_pool(name="psum_st", bufs=1, space="PSUM"))
        psum_bc = ctx.enter_context(tc.tile_pool(name="psum_bc", bufs=1, space="PSUM"))
        prod_pool = ctx.enter_context(tc.tile_pool(name="prod", bufs=2))
        t_pool = ctx.enter_context(tc.tile_pool(name="tpool", bufs=2))
        xt_pool = ctx.enter_context(tc.tile_pool(name="xtp", bufs=3))
        w_pool = ctx.enter_context(tc.tile_pool(name="wp", bufs=3))
        h_pool = ctx.enter_context(tc.tile_pool(name="hp", bufs=1))
        gate_pool = ctx.enter_context(tc.tile_pool(name="gatep", bufs=1))

        # -------- persistent tiles --------
        fps16 = persist.tile([P, 20, BC], MID)  # fps.T, ktile = enc*4 + ht
        prodw = persist.tile([P, 4, 10, BC], MID)  # pair products, [ht, pair]
        wqrep = persist.tile([P, 10, BC], MID)
        fpwrep = persist.tile([P, 5, BC], MID)
        mfallrep = persist.tile([P, BC], MID)
        common = persist.tile([P, 4, BC], MID)
        wsum = persist.tile([P, 4, BC], MID)
        enh_sb = persist.tile([P, 4, BC], MID)
        l5 = persist.tile([5, BC], MID)
        ones_col16 = persist.tile([P, 1], MID)
        ones_row16 = persist.tile([1, P], MID)
        pcat_sb = persist.tile([5, 10], MID)
        esel_sb = persist.tile([10, 10 * P], MID)
        dsel_sb = persist.tile([P, 15 * 15], MID)
        # PSUM accumulators: pair dots d (10 rows) and squared norms ss (5)
        d_ps = psum_acc.tile([10, BC], F32)
        ss_ps = psum_acc2.tile([5, BC], F32)
        d_n = [0]  # of 40 accumulating d matmuls
        ss_n = [0]  # of 20 accumulating ss matmuls
        biases = {}
        for name, _, K, dh in ENCS:
            biases[name] = (
                persist.tile([P, dh // P], F32, name=f"b1sb_{name}"),
                persist.tile([P, 4], F32, name=f"b2sb_{name}"),
            )
        wgb_sb = persist.tile([5, 1], F32)
        enhb_sb = persist.tile([P, 4], F32)
        fusb_sb = persist.tile([P, 4], F32)

        nc.vector.memset(ones_col16, 1.0)
        nc.vector.memset(ones_row16, 1.0)
        nc.sync.dma_start(pcat_sb, pcat.ap())
        nc.sync.dma_start(esel_sb, esel.ap())
        nc.sync.dma_start(dsel_sb, dsel.ap())
        for name, _, K, dh in ENCS:
            nc.sync.dma_start(biases[name][0], b1[name].ap())
            nc.sync.dma_start(biases[name][1], b2[name].ap())
        nc.sync.dma_start(wgb_sb, wg_b.ap())
        nc.sync.dma_start(enhb_sb, enh_b.ap())
        nc.sync.dma_start(fusb_sb, fus_b.ap())
        # prefetch all small weights used by phases B/D up front
        wg_sb = persist.tile([P, 20, 5], FP16)
        nc.sync.dma_start(wg_sb, wg_w.ap().rearrange("(ko p) m -> p ko m", p=P))
        ew_t = persist.tile([P, 4, 512], FP16, name="ew_t")
        nc.sync.dma_start(ew_t, enh_w.ap().rearrange("(ko p) m -> p ko m", p=P))
        fw_view = fus_w.ap().rearrange("(ko p) m -> p ko m", p=P)
        fw_lo = persist.tile([P, 4, 512], FP16, name="fw_lo")
        nc.sync.dma_start(fw_lo, fw_view[:, 0:4, :])
        fw_hi = persist.tile([P, 4, 512], FP16, name="fw_hi")
        nc.sync.dma_start(fw_hi, fw_view[:, 4:8, :])

        xt_view = xt.ap().rearrange("(ko p) n -> p ko n", p=P)

        # ================= Phase A: encoders (+ interleaved stats) =========
        def emit_d_group(p, engine):
            for ht in range(4):
                pr = prodw[:, ht, p, :]
                engine.tensor_mul(
                    pr, fps16[:, _I[p] * 4 + ht, :], fps16[:, _J[p] * 4 + ht, :]
                )
                # one-hot column p: reduction lands in d_ps row p
                nc.tensor.matmul(
                    d_ps,
                    dsel_sb[:, p * 15 : p * 15 + 10],
                    pr,
                    start=(d_n[0] == 0),
                    stop=(d_n[0] == 39),
                )
                d_n[0] += 1

        def emit_ss_group(i):
            for ht in range(4):
                sq = prod_pool.tile([P, BC], MID, tag="sq16")
                nc.scalar.square(sq, fps16[:, i * 4 + ht, :])
                nc.tensor.matmul(
                    ss_ps,
                    dsel_sb[:, i * 15 : i * 15 + 5],
                    sq,
                    start=(ss_n[0] == 0),
                    stop=(ss_n[0] == 19),
                )
                ss_n[0] += 1

        done_encs = []
        for name in ORDER:
            ei, (_, _, K, dh) = ENC_BY_NAME[name]
            M = dh // P
            b1_sb, b2_sb = biases[name]
            # ---- layer 1: h.T[dh, BC] = relu(w1.T @ x.T + b1) ----
            psums = [
                psum_mm.tile([P, BC], F32, tag="mmps", name=f"l1_{name}_{m}")
                for m in range(M)
            ]
            h_sb = h_pool.tile([P, 4, BC], MID, tag="htile")
            kdone = 0
            for kc0 in range(0, K, 4):
                kn = min(4, K - kc0)
                xt_t = xt_pool.tile([P, 4, BC], FP16, tag="xt")
                nc.sync.dma_start(
                    xt_t[:, :kn, :],
                    xt_view[:, XT_OFF[ei] + kc0 : XT_OFF[ei] + kc0 + kn, :],
                )
                w1_t = w_pool.tile([P, 4, 512], FP16, tag="w1")
                nc.sync.dma_start(
                    w1_t[:, :kn, :dh],
                    w1[name].ap()[kc0 * P : (kc0 + kn) * P, :].rearrange(
                        "(ko p) m -> p ko m", p=P
                    ),
                )
                for m in range(M):
                    for k in range(kn):
                        nc.tensor.matmul(
                            psums[m],
                            w1_t[:, k, m * P : (m + 1) * P],
                            xt_t[:, k, :],
                            start=(kdone + k == 0),
                            stop=(kdone + k == K - 1),
                        )
                kdone += kn
            for m in range(M):
                nc.scalar.activation(
                    h_sb[:, m, :], psums[m], AF.Relu, bias=b1_sb[:, m : m + 1]
                )
            # ---- layer 2: fps.T[H, BC] = w2.T @ h.T + b2 ----
            w2_t = w_pool.tile([P, 4, 512], FP16, tag="w1")
            nc.sync.dma_start(
                w2_t[:, :M, :], w2[name].ap().rearrange("(ko p) m -> p ko m", p=P)
            )
            for m in range(4):
                ps = psum_mm.tile([P, BC], F32, tag="mmps", name=f"l2_{name}_{m}")
                for k in range(M):
                    nc.tensor.matmul(
                        ps,
                        w2_t[:, k, m * P : (m + 1) * P],
                        h_sb[:, k, :],
                        start=(k == 0),
                        stop=(k == M - 1),
                    )
                nc.scalar.activation(
                    fps16[:, ei * 4 + m, :], ps, AF.Identity, bias=b2_sb[:, m : m + 1]
                )
            # ---- interleaved stats for this encoder + completed pairs ----
            emit_ss_group(ei)
            for prev in done_encs:
                pkey = (min(prev, ei), max(prev, ei))
                p = PAIR_IDX[pkey]
                # fp16 products are cheap on DVE, which is otherwise idle in
                # phase A (phase C's wide ops keep Pool busy later instead)
                emit_d_group(p, nc.vector)
            done_encs.append(ei)

        # ================= Phase B =========================================
        # z-gate matmuls first: they fill the PE while ACT/DVE walk the
        # d-stats softmax chain below
        z_ps = psum_st.tile([5, BC], F32, tag="stps", name="zgate")
        for kt in range(20):
            nc.tensor.matmul(
                z_ps, wg_sb[:, kt, :], fps16[:, kt, :], start=(kt == 0), stop=(kt == 19)
            )
        # ln of squared norms, then pairlog[p] = ln(ss_I) + ln(ss_J)
        nc.scalar.activation(l5, ss_ps, AF.Ln)
        pl_ps = psum_st.tile([10, BC], F32, tag="stps", name="pl")
        nc.tensor.matmul(pl_ps, pcat_sb, l5, start=True, stop=True)
        invnn = smalls.tile([10, BC], MID)  # 1/(norm_I*norm_J)
        nc.scalar.activation(invnn, pl_ps, AF.Exp, scale=-0.5)
        sims = smalls.tile([10, BC], MID)
        nc.vector.tensor_mul(sims, d_ps, invnn)
        e0 = smalls.tile([10, BC], MID)
        nc.scalar.activation(e0, sims, AF.Exp)
        e_sb = smalls.tile([10, BC], MID)
        # e = (d > 0) * exp(sims)
        nc.vector.scalar_tensor_tensor(
            e_sb, in0=d_ps, scalar=0.0, in1=e0, op0=ALU.is_gt, op1=ALU.mult
        )
        den_ps = psum_st.tile([1, BC], F32, tag="stps", name="den")
        nc.tensor.matmul(den_ps, ones_col16[0:10, :], e_sb, start=True, stop=True)
        # mean-fallback weight row: 0.2 * (1 - any(sel))
        mfr = smalls.tile([1, BC], MID)
        nc.vector.tensor_scalar(
            mfr, in0=den_ps, scalar1=0.0, scalar2=-0.2, op0=ALU.is_gt, op1=ALU.mult
        )
        mfr2 = smalls.tile([1, BC], MID)
        nc.vector.tensor_scalar_add(mfr2, mfr, 0.2)
        mfr = mfr2
        # 1/denom; denom is 0 (no sel) or > 1, so clamp at 1
        den_sb = smalls.tile([1, BC], F32)
        nc.vector.tensor_scalar_max(den_sb, den_ps, 1.0)
        recip32 = smalls.tile([1, BC], F32)
        nc.vector.reciprocal_approx_fast(recip32, den_sb)
        recip = smalls.tile([1, BC], MID)
        nc.scalar.activation(recip, recip32, AF.Copy)
        rr_ps = psum_st.tile([10, BC], F32, tag="stps", name="rr")
        nc.tensor.matmul(rr_ps, ones_row16[:, 0:10], recip, start=True, stop=True)
        wq_sb = smalls.tile([10, BC], MID)
        # wq = 0.5 * e / denom  (0.5 from the cf definition)
        nc.vector.scalar_tensor_tensor(
            wq_sb, in0=e_sb, scalar=0.5, in1=rr_ps, op0=ALU.mult, op1=ALU.mult
        )

        def broadcast(dst, src_tile, row, nm):
            # out[r, b] = sum_k esel[k, row*128+r] * src[k, b] = src[row, b]
            ksel = src_tile.shape[0]
            bc_ps = psum_bc.tile([P, BC], F32, tag="bcps", name=nm)
            nc.tensor.matmul(
                bc_ps,
                esel_sb[0:ksel, row * P : (row + 1) * P],
                src_tile,
                start=True,
                stop=True,
            )
            nc.scalar.activation(dst, bc_ps, AF.Copy)

        # learned per-fingerprint fusion weights fpw (softmax over 5)
        ez = smalls.tile([5, BC], MID)
        nc.scalar.activation(ez, z_ps, AF.Exp, bias=wgb_sb[0:5, :])
        sez_ps = psum_st.tile([1, BC], F32, tag="stps", name="sez")
        nc.tensor.matmul(sez_ps, ones_col16[0:5, :], ez, start=True, stop=True)
        sez_sb = smalls.tile([1, BC], F32, tag="lnrow", name="sez_sb")
        nc.vector.tensor_scalar_max(sez_sb, sez_ps, 1e-30)
        rez32 = smalls.tile([1, BC], F32)
        nc.vector.reciprocal_approx_fast(rez32, sez_sb)
        rez = smalls.tile([1, BC], MID)
        nc.scalar.activation(rez, rez32, AF.Copy)
        rz_ps = psum_st.tile([5, BC], F32, tag="stps", name="rz")
        nc.tensor.matmul(rz_ps, ones_row16[:, 0:5], rez, start=True, stop=True)
        fpw_sb = smalls.tile([5, BC], MID)
        nc.vector.tensor_mul(fpw_sb, ez, rz_ps)

        # wq broadcasts first: they unblock phase C's maskw
        for p in range(10):
            broadcast(wqrep[:, p, :], wq_sb, p, f"bc_wq{p}")
        broadcast(mfallrep, mfr, 0, "bc_mf")
        for i in range(5):
            broadcast(fpwrep[:, i, :], fpw_sb, i, f"bc_fpw{i}")

        # ================= Phase C: masked aggregation =====================
        fps_by_ht = fps16.rearrange("p (i h) n -> p h i n", h=4)
        for ht in range(4):
            # maskw_p = (prod_p > 0) * wq_p from the persistent pair products
            maskw = wide_pool.tile([P, 10, BC], MID, tag="prodw", name=f"maskw{ht}")
            nc.vector.scalar_tensor_tensor(
                maskw,
                in0=prodw[:, ht, :, :],
                scalar=0.0,
                in1=wqrep,
                op0=ALU.is_gt,
                op1=ALU.mult,
            )
            # G_i = sum of the 4 maskw of pairs containing i, + mean-fallback
            # (pure tree, no in-place RMW: in-place DVE adds run ~3x slower;
            # i 0-1 on the otherwise-idle Pool engine to offload the DVE)
            gs = gs_pool.tile([P, 5, BC], MID, tag="g")
            for i in range(5):
                eng = nc.gpsimd if i < 2 else nc.vector
                pa, pb, pc_, pd = PAIRS_OF[i]
                ga = t_pool.tile([P, BC], MID, tag="gtmp", name=f"ga{ht}_{i}")
                gb = t_pool.tile([P, BC], MID, tag="gtmp2", name=f"gb{ht}_{i}")
                gc = t_pool.tile([P, BC], MID, tag="gtmp3", name=f"gc{ht}_{i}")
                eng.tensor_add(ga, maskw[:, pa, :], maskw[:, pb, :])
                eng.tensor_add(gb, maskw[:, pc_, :], maskw[:, pd, :])
                eng.tensor_add(gc, ga, gb)
                eng.tensor_add(gs[:, i, :], gc, mfallrep)
            # common.T[ht] = sum_i fps_i.T * G_i  (wide mult + pair tree)
            tuw = wide_pool.tile([P, 10, BC], MID, tag="prodw", name=f"tuw{ht}")
            tw = tuw[:, 0:5, :]
            uw = tuw[:, 5:10, :]
            nc.vector.tensor_mul(tw, fps_by_ht[:, ht, :, :], gs)
            r1 = t_pool.tile([P, 2, BC], MID, tag="r1", name=f"r1_{ht}")
            nc.vector.tensor_add(r1, tw[:, 0:4:2, :], tw[:, 1:4:2, :])
            r2 = t_pool.tile([P, BC], MID, tag="gtmp", name=f"r2_{ht}")
            nc.vector.tensor_add(r2, r1[:, 0, :], r1[:, 1, :])
            nc.vector.tensor_add(common[:, ht, :], r2, tw[:, 4, :])
            # wsum.T[ht] likewise with the learned fusion weights
            nc.vector.tensor_mul(uw, fps_by_ht[:, ht, :, :], fpwrep)
            u1 = t_pool.tile([P, 2, BC], MID, tag="r1", name=f"u1_{ht}")
            nc.vector.tensor_add(u1, uw[:, 0:4:2, :], uw[:, 1:4:2, :])
            u2 = t_pool.tile([P, BC], MID, tag="gtmp2", name=f"u2_{ht}")
            nc.vector.tensor_add(u2, u1[:, 0, :], u1[:, 1, :])
            nc.vector.tensor_add(wsum[:, ht, :], u2, uw[:, 4, :])

        if dbg:
            nc.gpsimd.dma_start(dbg["fps16"].ap(), fps16)
            nc.sync.dma_start(dbg["stats"].ap(), stats)
            nc.sync.dma_start(dbg["ss"].ap(), ss_t)
            nc.gpsimd.dma_start(dbg["wq"].ap(), wq_sb)
            nc.gpsimd.dma_start(dbg["fpw"].ap(), fpw_sb)
            nc.gpsimd.dma_start(dbg["commonT"].ap(), common)
            nc.gpsimd.dma_start(dbg["wsumT"].ap(), wsum)
            nc.gpsimd.dma_start(dbg["wqrep"].ap(), wqrep)
            nc.gpsimd.dma_start(dbg["mfall"].ap(), mfallrep)

        # ================= Phase D: enhance + fuse =================
        for m in range(4):
            ps = psum_mm.tile([P, BC], F32, tag="mmps", name=f"enh_{m}")
            for k in range(4):
                nc.tensor.matmul(
                    ps,
                    ew_t[:, k, m * P : (m + 1) * P],
                    common[:, k, :],
                    start=(k == 0),
                    stop=(k == 3),
                )
            gate = gate_pool.tile([P, BC], MID, tag="gate")
            nc.scalar.activation(gate, ps, AF.Sigmoid, bias=enhb_sb[:, m : m + 1])
            nc.vector.tensor_mul(enh_sb[:, m, :], common[:, m, :], gate)

        out_view = out.ap().rearrange("(m p) n -> p m n", p=P)
        for m in range(4):
            ps = psum_mm.tile([P, BC], F32, tag="mmps", name=f"fus_{m}")
            for k in range(8):
                rhs = wsum[:, k, :] if k < 4 else enh_sb[:, k - 4, :]
                fw_t = fw_lo if k < 4 else fw_hi
                nc.tensor.matmul(
                    ps,
                    fw_t[:, k % 4, m * P : (m + 1) * P],
                    rhs,
                    start=(k == 0),
                    stop=(k == 7),
                )
            o_sb = gate_pool.tile([P, BC], F32, tag="osb")
            nc.scalar.activation(o_sb, ps, AF.Identity, bias=fusb_sb[:, m : m + 1])
            nc.sync.dma_start(out_view[:, m, :], o_sb)


def prep_inputs(inputs):
    """Host-side: build the per-core in_maps from full inputs."""
    x = np.asarray(inputs["fp_features"], np.float32)

    def pad_rows(a, rows):
        a = np.asarray(a, np.float32)
        if a.shape[0] == rows:
            return a
        out = np.zeros((rows, a.shape[1]), np.float32)
        out[: a.shape[0]] = a
        return out

    # padded transposed x, shared prep then per-core column slices
    xt_full = np.zeros((XT_K * P, B), np.float16)
    offs_in = np.cumsum([0, AP_D, MA_D, MB_D, MC_D])
    for ei, (name, din, K, dh) in enumerate(ENCS):
        seg = x[:, offs_in[ei] : offs_in[ei] + din]  # [B, din]
        xt_full[XT_OFF[ei] * P : XT_OFF[ei] * P + din, :] = seg.T.astype(np.float16)

    common_map = {}
    for ei, (name, din, K, dh) in enumerate(ENCS):
        common_map[f"w1_{name}"] = pad_rows(inputs[f"{name}_w1"], K * P).astype(
            np.float16
        )
        common_map[f"w2_{name}"] = np.asarray(inputs[f"{name}_w2"], np.float16)
        common_map[f"b1_{name}"] = (
            np.asarray(inputs[f"{name}_b1"], np.float32).reshape(dh // P, P).T.copy()
        )
        common_map[f"b2_{name}"] = (
            np.asarray(inputs[f"{name}_b2"], np.float32).reshape(4, P).T.copy()
        )
    common_map["wg_w"] = np.asarray(inputs["wg_w"], np.float32).astype(np.float16)
    common_map["wg_b"] = np.asarray(inputs["wg_b"], np.float32).reshape(5, 1)
    pcat = np.zeros((5, 10), np.float32)
    for p in range(10):
        pcat[_I[p], p] = 1.0
        pcat[_J[p], p] = 1.0
    common_map["pcat"] = pcat.astype(np.float16)
    esel = np.zeros((10, 10 * 128), np.float16)
    for p in range(10):
        esel[p, p * 128 : (p + 1) * 128] = 1.0
    common_map["esel"] = esel
    dsel_np = np.zeros((128, 15 * 15), np.float16)
    dsel_np[:, np.arange(15) * 16] = 1.0  # col g of group g
    common_map["dsel"] = dsel_np
    common_map["enh_w"] = np.asarray(inputs["enh_w"], np.float16)
    common_map["enh_b"] = np.asarray(inputs["enh_b"], np.float32).reshape(4, P).T.copy()
    common_map["fus_w"] = np.asarray(inputs["fus_w"], np.float16)
    common_map["fus_b"] = np.asarray(inputs["fus_b"], np.float32).reshape(4, P).T.copy()

    in_maps = []
    for c in range(N_CORES):
        m = dict(common_map)
        m["xt"] = np.ascontiguousarray(xt_full[:, c * BC : (c + 1) * BC])
        in_maps.append(m)
    return in_maps


_NC_CACHE = None


def kernel(**inputs) -> np.ndarray:
    global _NC_CACHE
    if _NC_CACHE is None:
        _NC_CACHE = build_bass()
    nc = _NC_CACHE
    in_maps = prep_inputs(inputs)
    res = run_bass_kernel_spmd(nc, in_maps, core_ids=list(range(N_CORES)))
    outs = [res.results[c]["out"] for c in range(N_CORES)]  # each [H, BC]
    full = np.concatenate([o.T for o in outs], axis=0)  # [B, H]
    return np.ascontiguousarray(full.astype(np.float32))





# revision 35
# speedup vs baseline: 2.3066x; 1.0646x over previous
"""Trainium2 Bass kernel for nn_CommonFeatureExtractor.

Data-parallel over 8 NeuronCores: batch dim (4096) sharded into 8 x 512,
weights replicated. Inside each core everything is computed in the
"transposed" layout [feature_on_partitions, batch_free] so that all matmul
contractions (which run over the partition axis on the PE) need no on-chip
transposes: the host feeds x already transposed and the weights are natural
[din, dout] = [K, M] layout, which is exactly what the PE's lhsT wants.

Pipeline per core (B=512 samples):
  A) 5 encoder MLPs (fp32 data, fp32r matmuls), fps.T stored bf16 [128,20,512]
  B) stats: pair products/squares (bf16) -> PE ones-matmul partition
     reductions -> d[10,B], ss[5,B]; softmax over selected pairs via
     ln/exp trick; per-pair weights broadcast to [128,B] via K=1 matmuls
  C) masked aggregation: G_i = sum_{pairs p containing i} (prod_p>0)*wq_p
     (+ mean-fallback), common.T = sum_i fps_i.T * G_i; wsum.T likewise with
     learned softmax gate weights
  D) enhance (sigmoid gate) + fuse matmuls -> fused.T [512, 512] -> host
     transposes back and concatenates.
"""

import numpy as np

import concourse.bass as bass
import concourse.mybir as mybir
import concourse.tile as tile
from concourse import bacc
from concourse.bass_utils import run_bass_kernel_spmd

F32 = mybir.dt.float32
F32R = mybir.dt.float32r
BF16 = mybir.dt.bfloat16
FP16 = mybir.dt.float16
ALU = mybir.AluOpType
AF = mybir.ActivationFunctionType

N_CORES = 8
B = 4096
BC = B // N_CORES  # 512 samples per core
H = 512
P = 128

AP_D, MA_D, MB_D, MC_D, PH_D = 2048, 167, 2048, 2048, 27
# encoders: (name, din, padded K tiles, hidden dh, M tiles = dh/128)
ENCS = [
    ("ap", AP_D, 16, 512),
    ("ma", MA_D, 2, 256),
    ("mb", MB_D, 16, 512),
    ("mc", MC_D, 16, 512),
    ("ph", PH_D, 1, 128),
]
XT_K = sum(e[2] for e in ENCS)  # 51 padded k-tiles of x
XT_OFF = np.cumsum([0] + [e[2] for e in ENCS])[:-1]  # [0,16,18,34,50]

_I = [0, 0, 0, 0, 1, 1, 1, 2, 2, 3]
_J = [1, 2, 3, 4, 2, 3, 4, 3, 4, 4]
PAIR_IDX = {(_I[p], _J[p]): p for p in range(10)}
# compute order: small encoders first so most pair-stats overlap phase A
ORDER = ["ma", "ph", "ap", "mb", "mc"]
ENC_BY_NAME = {e[0]: (i, e) for i, e in enumerate(ENCS)}
# pairs containing encoder i
PAIRS_OF = [[p for p in range(10) if _I[p] == i or _J[p] == i] for i in range(5)]

# midsection elementwise dtype
MID = FP16


DEBUG = False


def build_bass():
    nc = bacc.Bacc("TRN2", target_bir_lowering=False, debug=False)

    # ---------------- DRAM I/O ----------------
    xt = nc.dram_tensor("xt", [XT_K * P, BC], FP16, kind="ExternalInput")
    w1 = {}
    w2 = {}
    b1 = {}
    b2 = {}
    for name, _, K, dh in ENCS:
        w1[name] = nc.dram_tensor(f"w1_{name}", [K * P, dh], FP16, kind="ExternalInput")
        w2[name] = nc.dram_tensor(f"w2_{name}", [dh, H], FP16, kind="ExternalInput")
        b1[name] = nc.dram_tensor(f"b1_{name}", [P, dh // P], F32, kind="ExternalInput")
        b2[name] = nc.dram_tensor(f"b2_{name}", [P, 4], F32, kind="ExternalInput")
    wg_w = nc.dram_tensor("wg_w", [5 * H, 5], FP16, kind="ExternalInput")
    wg_b = nc.dram_tensor("wg_b", [5, 1], F32, kind="ExternalInput")
    pcat = nc.dram_tensor("pcat", [5, 10], FP16, kind="ExternalInput")
    esel = nc.dram_tensor("esel", [10, 10 * P], FP16, kind="ExternalInput")
    # one-hot column selectors: dsel[:, g*15:(g+1)*15] has column g all-ones,
    # so matmul(lhsT=dsel_g, rhs) reduces rhs over partitions into row g
    dsel = nc.dram_tensor("dsel", [P, 15 * 15], FP16, kind="ExternalInput")
    enh_w = nc.dram_tensor("enh_w", [H, H], FP16, kind="ExternalInput")
    enh_b = nc.dram_tensor("enh_b", [P, 4], F32, kind="ExternalInput")
    fus_w = nc.dram_tensor("fus_w", [2 * H, H], FP16, kind="ExternalInput")
    fus_b = nc.dram_tensor("fus_b", [P, 4], F32, kind="ExternalInput")
    out = nc.dram_tensor("out", [H, BC], F32, kind="ExternalOutput")
    dbg = {}
    if DEBUG:
        for nm, shape in [("fps16", [P, 20, BC]), ("stats", [10, BC]),
                          ("ss", [5, BC]), ("wq", [10, BC]), ("fpw", [5, BC]),
                          ("commonT", [P, 4, BC]), ("wsumT", [P, 4, BC]),
                          ("wqrep", [P, 10, BC]), ("mfall", [P, BC])]:
            dt = F32
            dbg[nm] = nc.dram_tensor(f"dbg_{nm}", shape, dt, kind="ExternalOutput")

    with tile.TileContext(nc) as tc:
        kernel_body(
            tc, xt, w1, w2, b1, b2, wg_w, wg_b, pcat, esel, dsel, enh_w, enh_b,
            fus_w, fus_b, out, dbg,
        )
    nc.compile()
    return nc


def kernel_body(
    tc, xt, w1, w2, b1, b2, wg_w, wg_b, pcat, esel, dsel, enh_w, enh_b, fus_w, fus_b,
    out, dbg={},
):
    nc = tc.nc

    import contextlib

    ctx = contextlib.ExitStack()
    with ctx:
        # -------- pools --------
        persist = ctx.enter_context(tc.tile_pool(name="persist", bufs=1))
        smalls = ctx.enter_context(tc.tile_pool(name="smalls", bufs=1))
        statrows = ctx.enter_context(tc.tile_pool(name="statrows", bufs=1))
        wide_pool = ctx.enter_context(tc.tile_pool(name="widep", bufs=2))
        gs_pool = ctx.enter_context(tc.tile_pool(name="gsp", bufs=1))
        psum_mm = ctx.enter_context(tc.tile_pool(name="psum_mm", bufs=4, space="PSUM"))
        psum_acc = ctx.enter_context(
            tc.tile_pool(name="psum_acc", bufs=1, space="PSUM")
        )
        psum_acc2 = ctx.enter_context(
            tc.tile_pool(name="psum_acc2", bufs=1, space="PSUM")
        )
        psum_st = ctx.enter_context(tc.tile_pool(name="psum_st", bufs=1, space="PSUM"))
        psum_bc = ctx.enter_context(tc.tile_pool(name="psum_bc", bufs=1, space="PSUM"))
        prod_pool = ctx.enter_context(tc.tile_pool(name="prod", bufs=2))
        t_pool = ctx.enter_context(tc.tile_pool(name="tpool", bufs=2))
        xt_pool = ctx.enter_context(tc.tile_pool(name="xtp", bufs=3))
        w_pool = ctx.enter_context(tc.tile_pool(name="wp", bufs=3))
        h_pool = ctx.enter_context(tc.tile_pool(name="hp", bufs=1))
        gate_pool = ctx.enter_context(tc.tile_pool(name="gatep", bufs=1))

        # -------- persistent tiles --------
        fps16 = persist.tile([P, 20, BC], MID)  # fps.T, ktile = enc*4 + ht
        prodw = persist.tile([P, 4, 10, BC], MID)  # pair products, [ht, pair]
        wqrep = persist.tile([P, 10, BC], MID)
        fpwrep = persist.tile([P, 5, BC], MID)
        mfallrep = persist.tile([P, BC], MID)
        common = persist.tile([P, 4, BC], MID)
        wsum = persist.tile([P, 4, BC], MID)
        enh_sb = persist.tile([P, 4, BC], MID)
        l5 = persist.tile([5, BC], MID)
        ones_col16 = persist.tile([P, 1], MID)
        ones_row16 = persist.tile([1, P], MID)
        pcat_sb = persist.tile([5, 10], MID)
        esel_sb = persist.tile([10, 10 * P], MID)
        dsel_sb = persist.tile([P, 15 * 15], MID)
        # PSUM accumulators: pair dots d (10 rows) and squared norms ss (5)
        d_ps = psum_acc.tile([10, BC], F32)
        ss_ps = psum_acc2.tile([5, BC], F32)
        d_n = [0]  # of 40 accumulating d matmuls
        ss_n = [0]  # of 20 accumulating ss matmuls
        biases = {}
        for name, _, K, dh in ENCS:
            biases[name] = (
                persist.tile([P, dh // P], F32, name=f"b1sb_{name}"),
                persist.tile([P, 4], F32, name=f"b2sb_{name}"),
            )
        wgb_sb = persist.tile([5, 1], F32)
        enhb_sb = persist.tile([P, 4], F32)
        fusb_sb = persist.tile([P, 4], F32)

        # minimal up-front loads: only what the first two (small) encoders
        # need, so their xt/w1 DMAs reach the head of the queue immediately
        nc.vector.memset(ones_col16, 1.0)
        nc.vector.memset(ones_row16, 1.0)
        nc.sync.dma_start(dsel_sb, dsel.ap())
        for name in ORDER[:2]:
            nc.sync.dma_start(biases[name][0], b1[name].ap())
            nc.sync.dma_start(biases[name][1], b2[name].ap())
        wg_sb = persist.tile([P, 20, 5], FP16)
        ew_t = persist.tile([P, 4, 512], FP16, name="ew_t")
        fw_view = fus_w.ap().rearrange("(ko p) m -> p ko m", p=P)
        fw_lo = persist.tile([P, 4, 512], FP16, name="fw_lo")
        fw_hi = persist.tile([P, 4, 512], FP16, name="fw_hi")

        def deferred_loads():
            # everything needed only by later encoders / phases B and D
            nc.sync.dma_start(pcat_sb, pcat.ap())
            nc.sync.dma_start(esel_sb, esel.ap())
            for name in ORDER[2:]:
                nc.sync.dma_start(biases[name][0], b1[name].ap())
                nc.sync.dma_start(biases[name][1], b2[name].ap())
            nc.sync.dma_start(wgb_sb, wg_b.ap())
            nc.sync.dma_start(enhb_sb, enh_b.ap())
            nc.sync.dma_start(fusb_sb, fus_b.ap())
            nc.sync.dma_start(wg_sb, wg_w.ap().rearrange("(ko p) m -> p ko m", p=P))
            nc.sync.dma_start(ew_t, enh_w.ap().rearrange("(ko p) m -> p ko m", p=P))
            nc.sync.dma_start(fw_lo, fw_view[:, 0:4, :])
            nc.sync.dma_start(fw_hi, fw_view[:, 4:8, :])

        xt_view = xt.ap().rearrange("(ko p) n -> p ko n", p=P)

        # ================= Phase A: encoders (+ interleaved stats) =========
        # Stat PE matmuls are DEFERRED into the middle of the NEXT encoder's
        # l1 stream: by then their DVE/ACT producers are long done, so the PE
        # never stalls mid-stream (stalls reset its p-state ramp to 1.2GHz).
        pending_mm = []

        def flush_stats():
            for which, g, rhs in pending_mm:
                if which == "d":
                    # one-hot column g: reduction lands in d_ps row g
                    nc.tensor.matmul(
                        d_ps,
                        dsel_sb[:, g * 15 : g * 15 + 10],
                        rhs,
                        start=(d_n[0] == 0),
                        stop=(d_n[0] == 39),
                    )
                    d_n[0] += 1
                else:
                    nc.tensor.matmul(
                        ss_ps,
                        dsel_sb[:, g * 15 : g * 15 + 5],
                        rhs,
                        start=(ss_n[0] == 0),
                        stop=(ss_n[0] == 19),
                    )
                    ss_n[0] += 1
            pending_mm.clear()

        def emit_d_group(p, engine):
            for ht in range(4):
                pr = prodw[:, ht, p, :]
                engine.tensor_mul(
                    pr, fps16[:, _I[p] * 4 + ht, :], fps16[:, _J[p] * 4 + ht, :]
                )
                pending_mm.append(("d", p, pr))

        def emit_ss_group(i):
            sq_t = prod_pool.tile([P, 4, BC], MID, tag="sq16", name=f"sq_{i}")
            for ht in range(4):
                nc.scalar.square(sq_t[:, ht, :], fps16[:, i * 4 + ht, :])
                pending_mm.append(("ss", i, sq_t[:, ht, :]))

        done_encs = []
        for name in ORDER:
            ei, (_, _, K, dh) = ENC_BY_NAME[name]
            M = dh // P
            b1_sb, b2_sb = biases[name]
            # ---- layer 1: h.T[dh, BC] = relu(w1.T @ x.T + b1) ----
            psums = [
                psum_mm.tile([P, BC], F32, tag="mmps", name=f"l1_{name}_{m}")
                for m in range(M)
            ]
            h_sb = h_pool.tile([P, 4, BC], MID, tag="htile")
            kdone = 0
            for kc0 in range(0, K, 4):
                kn = min(4, K - kc0)
                xt_t = xt_pool.tile([P, 4, BC], FP16, tag="xt")
                nc.sync.dma_start(
                    xt_t[:, :kn, :],
                    xt_view[:, XT_OFF[ei] + kc0 : XT_OFF[ei] + kc0 + kn, :],
                )
                w1_t = w_pool.tile([P, 4, 512], FP16, tag="w1")
                nc.sync.dma_start(
                    w1_t[:, :kn, :dh],
                    w1[name].ap()[kc0 * P : (kc0 + kn) * P, :].rearrange(
                        "(ko p) m -> p ko m", p=P
                    ),
                )
                for m in range(M):
                    for k in range(kn):
                        nc.tensor.matmul(
                            psums[m],
                            w1_t[:, k, m * P : (m + 1) * P],
                            xt_t[:, k, :],
                            start=(kdone + k == 0),
                            stop=(kdone + k == K - 1),
                        )
                kdone += kn
                if kc0 == 0:
                    flush_stats()  # previous encoder's stat matmuls
            for m in range(M):
                nc.scalar.activation(
                    h_sb[:, m, :], psums[m], AF.Relu, bias=b1_sb[:, m : m + 1]
                )
            # ---- layer 2: fps.T[H, BC] = w2.T @ h.T + b2 ----
            w2_t = w_pool.tile([P, 4, 512], FP16, tag="w1")
            nc.sync.dma_start(
                w2_t[:, :M, :], w2[name].ap().rearrange("(ko p) m -> p ko m", p=P)
            )
            for m in range(4):
                ps = psum_mm.tile([P, BC], F32, tag="mmps", name=f"l2_{name}_{m}")
                for k in range(M):
                    nc.tensor.matmul(
                        ps,
                        w2_t[:, k, m * P : (m + 1) * P],
                        h_sb[:, k, :],
                        start=(k == 0),
                        stop=(k == M - 1),
                    )
                nc.scalar.activation(
                    fps16[:, ei * 4 + m, :], ps, AF.Identity, bias=b2_sb[:, m : m + 1]
                )
            # ---- interleaved stats for this encoder + completed pairs ----
            emit_ss_group(ei)
            for prev in done_encs:
                pkey = (min(prev, ei), max(prev, ei))
                p = PAIR_IDX[pkey]
                # fp16 products are cheap on DVE, which is otherwise idle in
                # phase A (phase C's wide ops keep Pool busy later instead)
                emit_d_group(p, nc.vector)
            done_encs.append(ei)
            if name == ORDER[0]:
                deferred_loads()
        flush_stats()  # last encoder's stats run immediately

        # fsum[ht] = sum_i fps_i (for the mean-fallback term of common)
        fsum = persist.tile([P, 4, BC], MID)
        for ht in range(4):
            f_ht = fps_all_by_ht = fps16.rearrange("p (i h) n -> p h i n", h=4)
            t1 = t_pool.tile([P, BC], MID, tag="gtmp", name=f"fs1_{ht}")
            t2 = t_pool.tile([P, BC], MID, tag="gtmp2", name=f"fs2_{ht}")
            t3 = t_pool.tile([P, BC], MID, tag="gtmp3", name=f"fs3_{ht}")
            nc.vector.tensor_add(t1, f_ht[:, ht, 0, :], f_ht[:, ht, 1, :])
            nc.vector.tensor_add(t2, f_ht[:, ht, 2, :], f_ht[:, ht, 3, :])
            nc.vector.tensor_add(t3, t1, t2)
            nc.vector.tensor_add(fsum[:, ht, :], t3, f_ht[:, ht, 4, :])

        # ================= Phase B =========================================
        # z-gate matmuls first: they fill the PE while ACT/DVE walk the
        # d-stats softmax chain below
        z_ps = psum_st.tile([5, BC], F32, tag="stps", name="zgate")
        for kt in range(20):
            nc.tensor.matmul(
                z_ps, wg_sb[:, kt, :], fps16[:, kt, :], start=(kt == 0), stop=(kt == 19)
            )
        # ln of squared norms, then pairlog[p] = ln(ss_I) + ln(ss_J)
        nc.scalar.activation(l5, ss_ps, AF.Ln)
        pl_ps = psum_st.tile([10, BC], F32, tag="stps", name="pl")
        nc.tensor.matmul(pl_ps, pcat_sb, l5, start=True, stop=True)
        invnn = smalls.tile([10, BC], MID)  # 1/(norm_I*norm_J)
        nc.scalar.activation(invnn, pl_ps, AF.Exp, scale=-0.5)
        sims = smalls.tile([10, BC], MID)
        nc.vector.tensor_mul(sims, d_ps, invnn)
        e0 = smalls.tile([10, BC], MID)
        nc.scalar.activation(e0, sims, AF.Exp)
        e_sb = smalls.tile([10, BC], MID)
        # e = (d > 0) * exp(sims)
        nc.vector.scalar_tensor_tensor(
            e_sb, in0=d_ps, scalar=0.0, in1=e0, op0=ALU.is_gt, op1=ALU.mult
        )
        den_ps = psum_st.tile([1, BC], F32, tag="stps", name="den")
        nc.tensor.matmul(den_ps, ones_col16[0:10, :], e_sb, start=True, stop=True)
        # mean-fallback weight row: 0.2 * (1 - any(sel))
        mfr = smalls.tile([1, BC], MID)
        nc.vector.tensor_scalar(
            mfr, in0=den_ps, scalar1=0.0, scalar2=-0.2, op0=ALU.is_gt, op1=ALU.mult
        )
        mfr2 = smalls.tile([1, BC], MID)
        nc.vector.tensor_scalar_add(mfr2, mfr, 0.2)
        mfr = mfr2
        # 1/denom; denom is 0 (no sel) or > 1, so clamp at 1
        den_sb = smalls.tile([1, BC], F32)
        nc.vector.tensor_scalar_max(den_sb, den_ps, 1.0)
        recip32 = smalls.tile([1, BC], F32)
        nc.vector.reciprocal_approx_fast(recip32, den_sb)
        recip = smalls.tile([1, BC], MID)
        nc.scalar.activation(recip, recip32, AF.Copy)
        rr_ps = psum_st.tile([10, BC], F32, tag="stps", name="rr")
        nc.tensor.matmul(rr_ps, ones_row16[:, 0:10], recip, start=True, stop=True)
        wq_sb = smalls.tile([10, BC], MID)
        # wq = 0.5 * e / denom  (0.5 from the cf definition)
        nc.vector.scalar_tensor_tensor(
            wq_sb, in0=e_sb, scalar=0.5, in1=rr_ps, op0=ALU.mult, op1=ALU.mult
        )

        bc_idx = [0]

        def broadcast(dst, src_tile, row, nm):
            # out[r, b] = sum_k esel[k, row*128+r] * src[k, b] = src[row, b]
            # ping-pong across two PSUM banks so matmul N+1 overlaps copy N
            ksel = src_tile.shape[0]
            pool = psum_bc if bc_idx[0] % 2 == 0 else psum_st
            tag = "bcps" if bc_idx[0] % 2 == 0 else "stps"
            bc_idx[0] += 1
            bc_ps = pool.tile([P, BC], F32, tag=tag, name=nm)
            nc.tensor.matmul(
                bc_ps,
                esel_sb[0:ksel, row * P : (row + 1) * P],
                src_tile,
                start=True,
                stop=True,
            )
            nc.scalar.activation(dst, bc_ps, AF.Copy)

        # learned per-fingerprint fusion weights fpw (softmax over 5)
        ez = smalls.tile([5, BC], MID)
        nc.scalar.activation(ez, z_ps, AF.Exp, bias=wgb_sb[0:5, :])
        sez_ps = psum_st.tile([1, BC], F32, tag="stps", name="sez")
        nc.tensor.matmul(sez_ps, ones_col16[0:5, :], ez, start=True, stop=True)
        sez_sb = smalls.tile([1, BC], F32, tag="lnrow", name="sez_sb")
        nc.vector.tensor_scalar_max(sez_sb, sez_ps, 1e-30)
        rez32 = smalls.tile([1, BC], F32)
        nc.vector.reciprocal_approx_fast(rez32, sez_sb)
        rez = smalls.tile([1, BC], MID)
        nc.scalar.activation(rez, rez32, AF.Copy)
        rz_ps = psum_st.tile([5, BC], F32, tag="stps", name="rz")
        nc.tensor.matmul(rz_ps, ones_row16[:, 0:5], rez, start=True, stop=True)
        fpw_sb = smalls.tile([5, BC], MID)
        nc.vector.tensor_mul(fpw_sb, ez, rz_ps)

        # wq broadcasts first: they unblock phase C's maskw
        for p in range(10):
            broadcast(wqrep[:, p, :], wq_sb, p, f"bc_wq{p}")
        broadcast(mfallrep, mfr, 0, "bc_mf")
        for i in range(5):
            broadcast(fpwrep[:, i, :], fpw_sb, i, f"bc_fpw{i}")

        # mean-fallback term precomputed per ht: mfq = mfall * sum_i fps_i
        mfq = persist.tile([P, 4, BC], MID)
        for ht in range(4):
            nc.vector.tensor_mul(mfq[:, ht, :], mfallrep, fsum[:, ht, :])

        # ================= Phase C: masked aggregation =====================
        fps_by_ht = fps16.rearrange("p (i h) n -> p h i n", h=4)
        for ht in range(4):
            # maskw_p = (prod_p > 0) * wq_p from the persistent pair products
            maskw = wide_pool.tile([P, 10, BC], MID, tag="prodw", name=f"maskw{ht}")
            nc.vector.scalar_tensor_tensor(
                maskw,
                in0=prodw[:, ht, :, :],
                scalar=0.0,
                in1=wqrep,
                op0=ALU.is_gt,
                op1=ALU.mult,
            )
            # G_i = sum of the 4 maskw of pairs containing i (mean-fallback
            # folded in via mfq); i 0-1 on Pool to offload the DVE
            gs = gs_pool.tile([P, 5, BC], MID, tag="g")
            for i in range(5):
                eng = nc.gpsimd if i < 2 else nc.vector
                pa, pb, pc_, pd = PAIRS_OF[i]
                ga = t_pool.tile([P, BC], MID, tag="gtmp", name=f"ga{ht}_{i}")
                gb = t_pool.tile([P, BC], MID, tag="gtmp2", name=f"gb{ht}_{i}")
                eng.tensor_add(ga, maskw[:, pa, :], maskw[:, pb, :])
                eng.tensor_add(gb, maskw[:, pc_, :], maskw[:, pd, :])
                eng.tensor_add(gs[:, i, :], ga, gb)
            # common.T[ht] = sum_i fps_i.T * G_i + mfq  (wide mult + tree)
            tuw = wide_pool.tile([P, 10, BC], MID, tag="prodw", name=f"tuw{ht}")
            tw = tuw[:, 0:5, :]
            uw = tuw[:, 5:10, :]
            nc.vector.tensor_mul(tw, fps_by_ht[:, ht, :, :], gs)
            r1 = t_pool.tile([P, 2, BC], MID, tag="r1", name=f"r1_{ht}")
            nc.vector.tensor_add(r1, tw[:, 0:4:2, :], tw[:, 1:4:2, :])
            r2 = t_pool.tile([P, BC], MID, tag="gtmp", name=f"r2_{ht}")
            nc.vector.tensor_add(r2, r1[:, 0, :], r1[:, 1, :])
            r3 = t_pool.tile([P, BC], MID, tag="gtmp3", name=f"r3_{ht}")
            nc.gpsimd.tensor_add(r3, r2, tw[:, 4, :])
            nc.gpsimd.tensor_add(common[:, ht, :], r3, mfq[:, ht, :])
            # wsum.T[ht] likewise with the learned fusion weights
            nc.vector.tensor_mul(uw, fps_by_ht[:, ht, :, :], fpwrep)
            u1 = t_pool.tile([P, 2, BC], MID, tag="r1", name=f"u1_{ht}")
            nc.vector.tensor_add(u1, uw[:, 0:4:2, :], uw[:, 1:4:2, :])
            u2 = t_pool.tile([P, BC], MID, tag="gtmp2", name=f"u2_{ht}")
            nc.vector.tensor_add(u2, u1[:, 0, :], u1[:, 1, :])
            nc.vector.tensor_add(wsum[:, ht, :], u2, uw[:, 4, :])

        if dbg:
            nc.gpsimd.dma_start(dbg["fps16"].ap(), fps16)
            nc.sync.dma_start(dbg["stats"].ap(), stats)
            nc.sync.dma_start(dbg["ss"].ap(), ss_t)
            nc.gpsimd.dma_start(dbg["wq"].ap(), wq_sb)
            nc.gpsimd.dma_start(dbg["fpw"].ap(), fpw_sb)
            nc.gpsimd.dma_start(dbg["commonT"].ap(), common)
            nc.gpsimd.dma_start(dbg["wsumT"].ap(), wsum)
            nc.gpsimd.dma_start(dbg["wqrep"].ap(), wqrep)
            nc.gpsimd.dma_start(dbg["mfall"].ap(), mfallrep)

        # ================= Phase D: enhance + fuse =================
        for m in range(4):
            ps = psum_mm.tile([P, BC], F32, tag="mmps", name=f"enh_{m}")
            for k in range(4):
                nc.tensor.matmul(
                    ps,
                    ew_t[:, k, m * P : (m + 1) * P],
                    common[:, k, :],
                    start=(k == 0),
                    stop=(k == 3),
                )
            gate = gate_pool.tile([P, BC], MID, tag="gate")
            nc.scalar.activation(gate, ps, AF.Sigmoid, bias=enhb_sb[:, m : m + 1])
            nc.vector.tensor_mul(enh_sb[:, m, :], common[:, m, :], gate)

        out_view = out.ap().rearrange("(m p) n -> p m n", p=P)
        for m in range(4):
            ps = psum_mm.tile([P, BC], F32, tag="mmps", name=f"fus_{m}")
            for k in range(8):
                rhs = wsum[:, k, :] if k < 4 else enh_sb[:, k - 4, :]
                fw_t = fw_lo if k < 4 else fw_hi
                nc.tensor.matmul(
                    ps,
                    fw_t[:, k % 4, m * P : (m + 1) * P],
                    rhs,
                    start=(k == 0),
                    stop=(k == 7),
                )
            o_sb = gate_pool.tile([P, BC], F32, tag="osb")
            nc.scalar.activation(o_sb, ps, AF.Identity, bias=fusb_sb[:, m : m + 1])
            nc.sync.dma_start(out_view[:, m, :], o_sb)


def prep_inputs(inputs):
    """Host-side: build the per-core in_maps from full inputs."""
    x = np.asarray(inputs["fp_features"], np.float32)

    def pad_rows(a, rows):
        a = np.asarray(a, np.float32)
        if a.shape[0] == rows:
            return a
        out = np.zeros((rows, a.shape[1]), np.float32)
        out[: a.shape[0]] = a
        return out

    # padded transposed x, shared prep then per-core column slices
    xt_full = np.zeros((XT_K * P, B), np.float16)
    offs_in = np.cumsum([0, AP_D, MA_D, MB_D, MC_D])
    for ei, (name, din, K, dh) in enumerate(ENCS):
        seg = x[:, offs_in[ei] : offs_in[ei] + din]  # [B, din]
        xt_full[XT_OFF[ei] * P : XT_OFF[ei] * P + din, :] = seg.T.astype(np.float16)

    common_map = {}
    for ei, (name, din, K, dh) in enumerate(ENCS):
        common_map[f"w1_{name}"] = pad_rows(inputs[f"{name}_w1"], K * P).astype(
            np.float16
        )
        common_map[f"w2_{name}"] = np.asarray(inputs[f"{name}_w2"], np.float16)
        common_map[f"b1_{name}"] = (
            np.asarray(inputs[f"{name}_b1"], np.float32).reshape(dh // P, P).T.copy()
        )
        common_map[f"b2_{name}"] = (
            np.asarray(inputs[f"{name}_b2"], np.float32).reshape(4, P).T.copy()
        )
    common_map["wg_w"] = np.asarray(inputs["wg_w"], np.float32).astype(np.float16)
    common_map["wg_b"] = np.asarray(inputs["wg_b"], np.float32).reshape(5, 1)
    pcat = np.zeros((5, 10), np.float32)
    for p in range(10):
        pcat[_I[p], p] = 1.0
        pcat[_J[p], p] = 1.0
    common_map["pcat"] = pcat.astype(np.float16)
    esel = np.zeros((10, 10 * 128), np.float16)
    for p in range(10):
        esel[p, p * 128 : (p + 1) * 128] = 1.0
    common_map["esel"] = esel
    dsel_np = np.zeros((128, 15 * 15), np.float16)
    dsel_np[:, np.arange(15) * 16] = 1.0  # col g of group g
    common_map["dsel"] = dsel_np
    common_map["enh_w"] = np.asarray(inputs["enh_w"], np.float16)
    common_map["enh_b"] = np.asarray(inputs["enh_b"], np.float32).reshape(4, P).T.copy()
    common_map["fus_w"] = np.asarray(inputs["fus_w"], np.float16)
    common_map["fus_b"] = np.asarray(inputs["fus_b"], np.float32).reshape(4, P).T.copy()

    in_maps = []
    for c in range(N_CORES):
        m = dict(common_map)
        m["xt"] = np.ascontiguousarray(xt_full[:, c * BC : (c + 1) * BC])
        in_maps.append(m)
    return in_maps


_NC_CACHE = None


def kernel(**inputs) -> np.ndarray:
    global _NC_CACHE
    if _NC_CACHE is None:
        _NC_CACHE = build_bass()
    nc = _NC_CACHE
    in_maps = prep_inputs(inputs)
    res = run_bass_kernel_spmd(nc, in_maps, core_ids=list(range(N_CORES)))
    outs = [res.results[c]["out"] for c in range(N_CORES)]  # each [H, BC]
    full = np.concatenate([o.T for o in outs], axis=0)  # [B, H]
    return np.ascontiguousarray(full.astype(np.float32))



